# revision 1
# baseline (speedup 1.0000x reference)
"""Trainium2 Bass kernel for nn_DFlashAttentionSlide (GQA attention block).

Sharding: tensor-parallel over heads across 8 NeuronCores. Core c owns
kv head c and q heads [4c, 4c+4). Activations (x/x_ctx) are replicated;
weights / kv-cache are sharded along the head dim; the output projection
is contraction-sharded, so each core returns a partial [L, HID] output
that the host sums.

Device-side layout strategy (per core):
  - projections:  q as [l, hd] (N=512 matmuls), k/v as [d, t] (N=512)
  - attention scores computed TRANSPOSED: scoresT[s, (h l)] = K @ Q^T
    with k tiles as the stationary operand and all 4 heads' q packed in
    the 512-wide moving operand.  The PV matmul consumes the exp tiles
    directly (contraction over s = partition dim) producing outT
    [d, (h l)] -- no probability transposes anywhere.
  - the causal mask is applied MULTIPLICATIVELY after exp: exp(s+m) =
    exp(s)*exp(m), with exp(mask) precomputed on the host and
    head-replicated on device, so the s-loop mask op is a flat bf16 mul.
  - RMSNorm mean-subtract is folded into the projection weights on the
    host; variance uses sum-of-squares via ones-matmul partition
    reductions; rstd broadcast back across partitions with a K=1
    ones-matmul.
  - RoPE rotate-half is a cross-partition move done with two SBUF->SBUF
    DMA copies; the sign flip is folded into host-built sin tables.
    SCALE (1/sqrt(D)) is folded into the q-side cos/sin tables.
  - all HBM traffic runs on the hardware-DGE (sync) ring; resident
    tensors (kv cache halves, mask, tables, Wo) are chunked and
    interleaved into the projection stream so the PE-feeding cT tiles
    keep queue priority.  GPSIMD does elementwise work only.
"""

import os
import sys

sys.path.insert(0, "/opt/trn_rl_repo")

import numpy as np
import ml_dtypes

import concourse.bass as bass
import concourse.bacc as bacc
import concourse.tile as tile
from concourse import mybir
from concourse.bass_utils import run_bass_kernel_spmd

BF16 = ml_dtypes.bfloat16

H, HKV, D, HALF = 32, 8, 128, 64
L, T, S, HID = 128, 1024, 4096, 4096
REP = H // HKV          # q heads per kv head (= per core)
EPS = 1e-6
SCALE = D ** -0.5
NCORES = 8
KT = HID // 128         # 32 contraction tiles for projections
ST = S // 128           # 32 s tiles for attention
SOLD = S - T            # 3072 cached stream positions kept
TNEW = T                # 1024 newly projected stream positions

FP32 = mybir.dt.float32
BF16_DT = mybir.dt.bfloat16

_PROGRAM_CACHE = {}

# Filled by kernel() when BASS_KERNEL_TRACE=1; read by test.py.
LAST_RESULTS = None


def _build_program():
    nc = bacc.Bacc("TRN2", target_bir_lowering=False, debug=False,
                   num_devices=NCORES)

    # ---- external I/O (per-core values supplied via in_maps) ----
    cT = nc.declare_dram_parameter("cT", [HID, T], BF16_DT, isOutput=False)
    wkvT = nc.declare_dram_parameter("wkvT", [HID, 256], BF16_DT, isOutput=False)
    wqT = nc.declare_dram_parameter("wqT", [HID, 512], BF16_DT, isOutput=False)
    xTp = nc.declare_dram_parameter("xTp", [128, KT * 128], BF16_DT, isOutput=False)
    woP = nc.declare_dram_parameter("woP", [128, HID // 512, REP, 512], BF16_DT, isOutput=False)
    ktold = nc.declare_dram_parameter("ktold", [D, SOLD], BF16_DT, isOutput=False)
    voldP = nc.declare_dram_parameter("voldP", [128, SOLD], BF16_DT, isOutput=False)
    identf = nc.declare_dram_parameter("identf", [128, 128], FP32, isOutput=False)
    identb2 = nc.declare_dram_parameter("identb2", [128, 128], BF16_DT, isOutput=False)
    maskT = nc.declare_dram_parameter("maskT", [128, S], BF16_DT, isOutput=False)
    cosq = nc.declare_dram_parameter("cosq", [D, L], FP32, isOutput=False)
    sinq = nc.declare_dram_parameter("sinq", [D, L], FP32, isOutput=False)
    cosk = nc.declare_dram_parameter("cosk", [D, TNEW], FP32, isOutput=False)
    sink = nc.declare_dram_parameter("sink", [D, TNEW], FP32, isOutput=False)
    qw = nc.declare_dram_parameter("qw", [D, 1], FP32, isOutput=False)
    kw = nc.declare_dram_parameter("kw", [D, 1], FP32, isOutput=False)
    y = nc.declare_dram_parameter("y", [L, HID], FP32, isOutput=True)

    with tile.TileContext(nc) as tc:
        _emit(nc, tc, cT=cT, wkvT=wkvT, wqT=wqT, xTp=xTp, woP=woP, ktold=ktold, voldP=voldP,
              identf=identf, identb2=identb2,
              maskT=maskT, cosq=cosq, sinq=sinq, cosk=cosk, sink=sink,
              qw=qw, kw=kw, y=y)
    nc.compile()
    return nc


def _emit(nc, tc, *, cT, wkvT, wqT, xTp, woP, ktold, voldP, identf, identb2,
          maskT, cosq, sinq, cosk, sink, qw, kw, y):
    from contextlib import ExitStack
    from concourse.masks import make_identity

    ctx = ExitStack()
    with ctx:
        # ---------------- pools ----------------
        consts = ctx.enter_context(tc.tile_pool(name="consts", bufs=1))
        streams = ctx.enter_context(tc.tile_pool(name="streams", bufs=1))
        proj_in = ctx.enter_context(tc.tile_pool(name="proj_in", bufs=6))
        normtmp = ctx.enter_context(tc.tile_pool(name="normtmp", bufs=1))
        sloop = ctx.enter_context(tc.tile_pool(name="sloop", bufs=4))
        psA = ctx.enter_context(tc.tile_pool(name="psA", bufs=1, space="PSUM"))
        psS = ctx.enter_context(tc.tile_pool(name="psS", bufs=3, space="PSUM"))

        # ---------------- constants (no DMA) ----------------
        ones_col = consts.tile([128, 1], FP32, tag="ones_col")
        nc.vector.memset(ones_col, 1.0)
        ones_colb = consts.tile([128, 1], BF16_DT, tag="ones_colb")
        nc.vector.memset(ones_colb, 1.0)
        ones_row = consts.tile([1, 128], FP32, tag="ones_row")
        nc.vector.memset(ones_row, 1.0)
        eps_t = consts.tile([128, 1], FP32, tag="eps")
        nc.vector.memset(eps_t, EPS)
        ident = consts.tile([128, 128], FP32, tag="ident")
        nc.sync.dma_start(ident[:], identf[:])
        identb = consts.tile([128, 128], BF16_DT, tag="identb")
        nc.sync.dma_start(identb[:], identb2[:])

        # ---------------- resident tiles (DMAs interleaved below) --------
        kts = streams.tile([128, S], BF16_DT, tag="kts")
        vt = streams.tile([128, S], BF16_DT, tag="vt")
        mask_all = streams.tile([128, S], BF16_DT, tag="mask")
        mask4 = streams.tile([128, ST, REP, 128], BF16_DT, tag="mask4")
        wo_res = streams.tile([128, HID // 512, REP, 512], BF16_DT, tag="wo")
        qw_t = consts.tile([D, 1], FP32, tag="qw")
        kw_t = consts.tile([D, 1], FP32, tag="kw")
        cosq_t = consts.tile([D, L], FP32, tag="cosq")
        sinq_t = consts.tile([D, L], FP32, tag="sinq")
        cosk_t = consts.tile([D, TNEW], FP32, tag="cosk")
        sink_t = consts.tile([D, TNEW], FP32, tag="sink")



        xT_res = streams.tile([128, KT * 128], BF16_DT, tag="xT")

        def resident_chunk(k):
            # early-needed resident loads only (kts/vt/mask/tables), split
            # across the two HWDGE queues; wo loads happen during the s-loop
            if k < 8:  # kts old: 8 x [128, 384] on qSP
                nc.sync.dma_start(kts[:, k * 384:(k + 1) * 384],
                                  ktold[:, k * 384:(k + 1) * 384])
            if 24 <= k < 32:  # x.T for the q projection: 8 x [128, 512]
                j = k - 24
                nc.sync.dma_start(xT_res[:, j * 512:(j + 1) * 512],
                                  xTp[:, j * 512:(j + 1) * 512])
            if k < 24:  # v old (host-packed): 24 x [128, 128] contiguous
                nc.scalar.dma_start(vt[:, k * 128:(k + 1) * 128],
                                    voldP[:, k * 128:(k + 1) * 128])
            if 8 <= k < 16:  # mask: 8 x [128, 512] on qSP
                j = k - 8
                nc.sync.dma_start(mask_all[:, j * 512:(j + 1) * 512],
                                  maskT[:, j * 512:(j + 1) * 512])
            if 16 <= k < 24:  # rope tables + norm weights on qSP
                j = k - 16
                if j < 2:
                    nc.sync.dma_start(cosk_t[:, j * 512:(j + 1) * 512],
                                      cosk[:, j * 512:(j + 1) * 512])
                elif j < 4:
                    jj = j - 2
                    nc.sync.dma_start(sink_t[:, jj * 512:(jj + 1) * 512],
                                      sink[:, jj * 512:(jj + 1) * 512])
                elif j == 4:
                    nc.sync.dma_start(cosq_t[:], cosq[:])
                elif j == 5:
                    nc.sync.dma_start(sinq_t[:], sinq[:])
                elif j == 6:
                    nc.sync.dma_start(qw_t[:], qw[:])
                else:
                    nc.sync.dma_start(kw_t[:], kw[:])

        # ---------------- projections ----------------
        ps_q = psA.tile([128, 512], FP32, tag="ps_q")
        ps_k0 = psA.tile([128, 512], FP32, tag="ps_k0")
        ps_k1 = psA.tile([128, 512], FP32, tag="ps_k1")
        ps_v0 = psA.tile([128, 512], FP32, tag="ps_v0")
        ps_v1 = psA.tile([128, 512], FP32, tag="ps_v1")

        with nc.named_scope("proj"):
            for k in range(KT):
                ct_k = proj_in.tile([128, T], BF16_DT, tag="ct")
                nc.sync.dma_start(ct_k[:], cT[k * 128:(k + 1) * 128, :])
                w_k = proj_in.tile([128, 256], BF16_DT, tag="wkv")
                nc.scalar.dma_start(w_k[:], wkvT[k * 128:(k + 1) * 128, :])
                resident_chunk(k)

                st = (k == 0)
                sp = (k == KT - 1)
                nc.tensor.matmul(ps_k0[:], w_k[:, 0:128], ct_k[:, 0:512],
                                 start=st, stop=sp)
                nc.tensor.matmul(ps_k1[:], w_k[:, 0:128], ct_k[:, 512:1024],
                                 start=st, stop=sp)
                nc.tensor.matmul(ps_v0[:], w_k[:, 128:256], ct_k[:, 0:512],
                                 start=st, stop=sp)
                nc.tensor.matmul(ps_v1[:], w_k[:, 128:256], ct_k[:, 512:1024],
                                 start=st, stop=sp)
            # q projection against the resident x.T (overlaps the k/v norm)
            for k in range(KT):
                wq_k = proj_in.tile([128, 512], BF16_DT, tag="wq")
                nc.scalar.dma_start(wq_k[:], wqT[k * 128:(k + 1) * 128, :])
                nc.tensor.matmul(ps_q[:], xT_res[:, k * 128:(k + 1) * 128],
                                 wq_k[:], start=(k == 0), stop=(k == KT - 1))

        # head-replicate the multiplicative mask (GPSIMD elementwise copy)
        m2d = mask_all[:].rearrange("p (s l) -> p s l", l=128)
        for r in range(REP):
            nc.vector.tensor_copy(mask4[:, :, r, :], m2d)

        with nc.named_scope("norm"):
            # copy accumulators out on ACT (idle here); frees proj banks
            qsb = normtmp.tile([128, 512], FP32, tag="qsb")
            nc.scalar.copy(qsb[:], ps_q[:])
            kc = normtmp.tile([128, TNEW], FP32, tag="kc")
            nc.scalar.copy(kc[:, 0:512], ps_k0[:])
            nc.scalar.copy(kc[:, 512:1024], ps_k1[:])
            vsb = normtmp.tile([128, TNEW], BF16_DT, tag="vsb")
            nc.scalar.copy(vsb[:, 0:512], ps_v0[:])
            nc.scalar.copy(vsb[:, 512:1024], ps_v1[:])

            # ---- q rmsnorm + rope (first: unblocks the attention loop) ----
            qsq = normtmp.tile([128, 512], FP32, tag="qsq")
            nc.vector.tensor_mul(qsq[:], qsb[:], qsb[:])
            qsos = normtmp.tile([128, REP], FP32, tag="qsos")
            nc.vector.reduce_sum(
                qsos[:],
                qsq[:].rearrange("p (h l) -> p h l", h=REP),
                axis=mybir.AxisListType.X,
            )
            qrstd = normtmp.tile([128, REP], FP32, tag="qrstd")
            nc.scalar.activation(qrstd[:], qsos[:],
                                 mybir.ActivationFunctionType.Sqrt,
                                 bias=eps_t[:], scale=1.0 / D)
            nc.vector.reciprocal(qrstd[:], qrstd[:])
            qn = normtmp.tile([128, 512], FP32, tag="qn")
            for h in range(REP):
                nc.vector.tensor_scalar_mul(qn[:, h * 128:(h + 1) * 128],
                                            qsb[:, h * 128:(h + 1) * 128],
                                            qrstd[:, h:h + 1])
            qT_all = streams.tile([128, 512], BF16_DT, tag="qT_all")
            qtw = normtmp.tile([128, 512], FP32, tag="qtw")
            for h in range(REP):
                ps_qT = psA.tile([128, 128], FP32, tag="ps_q")
                nc.tensor.transpose(ps_qT[:], qn[:, h * 128:(h + 1) * 128],
                                    ident[:])
                nc.vector.tensor_scalar_mul(qtw[:, h * 128:(h + 1) * 128],
                                            ps_qT[:], qw_t[:])
            qrot = normtmp.tile([128, 512], FP32, tag="qrot")
            nc.sync.dma_start(qrot[0:HALF, :], qtw[HALF:D, :])
            nc.sync.dma_start(qrot[HALF:D, :], qtw[0:HALF, :])
            qa = normtmp.tile([128, 512], FP32, tag="qsq")
            qb = normtmp.tile([128, 512], FP32, tag="qn")
            for h in range(REP):
                sl = slice(h * 128, (h + 1) * 128)
                nc.vector.tensor_mul(qa[:, sl], qtw[:, sl], cosq_t[:])
                nc.vector.tensor_mul(qb[:, sl], qrot[:, sl], sinq_t[:])
            nc.vector.tensor_add(qT_all[:], qa[:], qb[:])

            # ---- k rmsnorm (mean already folded into weights) + rope ----
            ksq = normtmp.tile([128, TNEW], FP32, tag="ksq")
            nc.vector.tensor_mul(ksq[:, 0:512], kc[:, 0:512], kc[:, 0:512])
            nc.vector.tensor_mul(ksq[:, 512:1024], kc[:, 512:1024],
                                 kc[:, 512:1024])
            ps_sos0 = psA.tile([1, 512], FP32, tag="ps_k0")
            ps_sos1 = psA.tile([1, 512], FP32, tag="ps_k1")
            nc.tensor.matmul(ps_sos0[:], ones_col[:], ksq[:, 0:512])
            nc.tensor.matmul(ps_sos1[:], ones_col[:], ksq[:, 512:1024])
            krstd = normtmp.tile([1, TNEW], FP32, tag="krstd")
            nc.scalar.activation(krstd[:, 0:512], ps_sos0[:],
                                 mybir.ActivationFunctionType.Sqrt,
                                 bias=eps_t[0:1, :], scale=1.0 / D)
            nc.scalar.activation(krstd[:, 512:1024], ps_sos1[:],
                                 mybir.ActivationFunctionType.Sqrt,
                                 bias=eps_t[0:1, :], scale=1.0 / D)
            nc.vector.reciprocal(krstd[:], krstd[:])
            ps_krb0 = psA.tile([128, 512], FP32, tag="ps_k0")
            ps_krb1 = psA.tile([128, 512], FP32, tag="ps_k1")
            nc.tensor.matmul(ps_krb0[:], ones_row[:], krstd[:, 0:512])
            nc.tensor.matmul(ps_krb1[:], ones_row[:], krstd[:, 512:1024])
            knw = normtmp.tile([128, TNEW], FP32, tag="knw")
            nc.vector.scalar_tensor_tensor(knw[:, 0:512], kc[:, 0:512],
                                           kw_t[:], ps_krb0[:],
                                           op0=mybir.AluOpType.mult,
                                           op1=mybir.AluOpType.mult)
            nc.vector.scalar_tensor_tensor(knw[:, 512:1024], kc[:, 512:1024],
                                           kw_t[:], ps_krb1[:],
                                           op0=mybir.AluOpType.mult,
                                           op1=mybir.AluOpType.mult)
            krot = normtmp.tile([128, TNEW], FP32, tag="krot")
            nc.sync.dma_start(krot[0:HALF, :], knw[HALF:D, :])
            nc.sync.dma_start(krot[HALF:D, :], knw[0:HALF, :])
            ka = normtmp.tile([128, TNEW], FP32, tag="ksq")
            nc.vector.tensor_mul(ka[:], knw[:], cosk_t[:])
            kb = normtmp.tile([128, TNEW], FP32, tag="kb")
            nc.vector.tensor_mul(kb[:], krot[:], sink_t[:])
            nc.vector.tensor_add(kts[:, SOLD:S], ka[:], kb[:])

            # ---- v transpose into stream tiles via PE ----
            for i in range(TNEW // 128):
                ps_vT = psA.tile([128, 128], BF16_DT, tag="ps_v1")
                nc.tensor.transpose(ps_vT[:], vsb[:, i * 128:(i + 1) * 128],
                                    identb[:])
                nc.vector.tensor_copy(
                    vt[:, SOLD + i * 128:SOLD + (i + 1) * 128], ps_vT[:])

        # ---------------- attention s-loop ----------------
        ps_o = psA.tile([128, 512], FP32, tag="ps_v0")
        ps_sum = psA.tile([1, 512], FP32, tag="ps_q")
        with nc.named_scope("sloop"):
            for s in range(ST):
                if s % 4 == 0:  # wo (host-packed): 8 contiguous 512KB chunks
                    j = s // 4
                    nc.sync.dma_start(wo_res[:, j, :, :], woP[:, j, :, :])
                ps_sc = psS.tile([128, 512], FP32, tag="ps_sc")
                nc.tensor.matmul(ps_sc[:], kts[:, s * 128:(s + 1) * 128],
                                 qT_all[:])
                scb = sloop.tile([128, 512], FP32, tag="scb")
                nc.vector.tensor_copy(scb[:], ps_sc[:])
                exr = sloop.tile([128, 512], BF16_DT, tag="exr")
                nc.scalar.activation(exr[:], scb[:],
                                     mybir.ActivationFunctionType.Exp)
                ex = sloop.tile([128, 512], BF16_DT, tag="ex")
                nc.vector.tensor_mul(
                    ex[:], exr[:],
                    mask4[:, s, :, :].rearrange("p h l -> p (h l)"))
                nc.tensor.matmul(ps_sum[:], ones_colb[:], ex[:],
                                 start=(s == 0), stop=(s == ST - 1))
                nc.tensor.matmul(ps_o[:], vt[:, s * 128:(s + 1) * 128], ex[:],
                                 start=(s == 0), stop=(s == ST - 1))

        # ---------------- normalize ----------------
        with nc.named_scope("fin"):
            rec = normtmp.tile([1, 512], FP32, tag="rec")
            nc.vector.reciprocal(rec[:], ps_sum[:])
            ps_rb = psA.tile([128, 512], FP32, tag="ps_k0")
            nc.tensor.matmul(ps_rb[:], ones_row[:], rec[:])
            osb = normtmp.tile([128, 512], FP32, tag="osb")
            nc.scalar.copy(osb[:], ps_o[:])
            attT = streams.tile([128, 512], BF16_DT, tag="attT")
            nc.vector.tensor_mul(attT[:], osb[:], ps_rb[:])

        # ---------------- output projection (partial) ----------------
        with nc.named_scope("oproj"):
            for e in range(HID // 512):
                ps_y = psA.tile([128, 512], FP32,
                                tag=("ps_k1" if e % 2 else "ps_v0"))
                for h in range(REP):
                    nc.tensor.matmul(
                        ps_y[:], attT[:, h * 128:(h + 1) * 128],
                        wo_res[:, e, h, :],
                        start=(h == 0), stop=(h == REP - 1))
                ysb = sloop.tile([128, 512], FP32, tag="ysb")
                nc.vector.tensor_copy(ysb[:], ps_y[:])
                nc.sync.dma_start(y[:, e * 512:(e + 1) * 512], ysb[:])


def _prepare_inputs(x, x_ctx, cos_q, sin_q, cos_k, sin_k, kv_cache,
                    causal_mask, Wq, Wk, Wv, Wo, q_norm_w, k_norm_w):
    """Host-side sharding/preprocessing. Returns list of per-core in_maps."""
    f32 = np.float32
    x = np.asarray(x, f32)
    x_ctx = np.asarray(x_ctx, f32)
    c = np.concatenate([x_ctx[0], x[0]], axis=0)          # [T, HID]
    cT = np.ascontiguousarray(c.T).astype(BF16)           # [HID, T]

    # x.T packed [p, (k 128l)]: xTp[p, k*128+l] = c.T[k*128+p, T-L+l]
    xTp = np.ascontiguousarray(
        c.T[:, T - L:T].reshape(KT, 128, L).transpose(1, 0, 2)
        .reshape(128, KT * L)).astype(BF16)

    m = np.asarray(causal_mask, f32)[0, 0]                # [L, S]
    # multiplicative mask exp(m), packed [s_local, (s_tile l)]
    maskP = np.ascontiguousarray(np.exp(
        m.T.reshape(S // 128, 128, L).transpose(1, 0, 2).reshape(128, S)))

    cosqT = np.ascontiguousarray(np.asarray(cos_q, f32)[0, 0].T) * SCALE
    sinqT = np.ascontiguousarray(np.asarray(sin_q, f32)[0, 0].T).copy()
    sinqT[:HALF] = -sinqT[:HALF]
    sinqT *= SCALE
    coskT = np.ascontiguousarray(np.asarray(cos_k, f32)[0, 0].T)
    sinkT = np.ascontiguousarray(np.asarray(sin_k, f32)[0, 0].T).copy()
    sinkT[:HALF] = -sinkT[:HALF]

    qwc = np.ascontiguousarray(np.asarray(q_norm_w, f32).reshape(D, 1))
    kwc = np.ascontiguousarray(np.asarray(k_norm_w, f32).reshape(D, 1))

    Wq = np.asarray(Wq, f32)
    Wk = np.asarray(Wk, f32)
    Wv = np.asarray(Wv, f32)
    Wo = np.asarray(Wo, f32)
    kv = np.asarray(kv_cache, f32)

    in_maps = []
    for cidx in range(NCORES):
        hd = slice(cidx * REP * D, (cidx + 1) * REP * D)
        wq_c = Wq[hd].reshape(REP, D, HID)
        wq_c = wq_c - wq_c.mean(axis=1, keepdims=True)    # fold mean-subtract
        wq_c = wq_c.reshape(REP * D, HID)
        wk_c = Wk[cidx * D:(cidx + 1) * D]
        wk_c = wk_c - wk_c.mean(axis=0, keepdims=True)
        wv_c = Wv[cidx * D:(cidx + 1) * D]
        wkvT = np.concatenate([wk_c.T, wv_c.T], axis=1)   # [HID, 256]
        wqTc = np.ascontiguousarray(wq_c.T)               # [HID, 512]
        # wo packed [p, e_chunk, h, 512]: woP[p,j,h,e'] = Wo.T[h*128+p, j*512+e']
        woTc = Wo[:, hd].T.reshape(REP, 128, HID // 512, 512)
        woP = np.ascontiguousarray(woTc.transpose(1, 2, 0, 3))
        ktold = np.ascontiguousarray(kv[0, cidx, T:, :].T)  # [D, SOLD]
        # vold packed [s_local, (tile d)]: voldP[p, n*128+d] = v[n*128+p, d]
        voldP = np.ascontiguousarray(
            kv[1, cidx, T:, :].reshape(SOLD // 128, 128, D)
            .transpose(1, 0, 2).reshape(128, SOLD))
        in_maps.append(dict(
            cT=cT,
            wkvT=np.ascontiguousarray(wkvT).astype(BF16),
            wqT=wqTc.astype(BF16),
            xTp=xTp,
            woP=woP.astype(BF16),
            ktold=ktold.astype(BF16),
            voldP=voldP.astype(BF16),
            identf=np.eye(128, dtype=f32),
            identb2=np.eye(128, dtype=f32).astype(BF16),
            maskT=maskP.astype(BF16),
            cosq=cosqT.astype(f32), sinq=sinqT.astype(f32),
            cosk=coskT.astype(f32), sink=sinkT.astype(f32),
            qw=qwc, kw=kwc,
        ))
    return in_maps


def kernel(**inputs) -> np.ndarray:
    global LAST_RESULTS
    if "nc" not in _PROGRAM_CACHE:
        _PROGRAM_CACHE["nc"] = _build_program()
    nc = _PROGRAM_CACHE["nc"]
    in_maps = _prepare_inputs(**inputs)
    trace = bool(int(os.environ.get("BASS_KERNEL_TRACE", "0")))
    res = run_bass_kernel_spmd(nc, in_maps, list(range(NCORES)), trace=trace)
    LAST_RESULTS = res
    y = np.zeros((L, HID), np.float64)
    for cidx in range(NCORES):
        y += res.results[cidx]["y"].astype(np.float64)
    return y.astype(np.float32).reshape(1, L, HID)



# revision 10
# speedup vs baseline: 1.1667x; 1.1667x over previous
"""Trainium2 Bass kernel for nn_DFlashAttentionSlide (GQA attention block).

Sharding: tensor-parallel over heads across 8 NeuronCores. Core c owns
kv head c and q heads [4c, 4c+4). Activations (x/x_ctx) are replicated;
weights / kv-cache are sharded along the head dim; the output projection
is contraction-sharded, so each core returns a partial [L, HID] output
that the host sums.

Device-side schedule (per core), built so the PE array streams matmuls
back-to-back while the big cT DMA overlaps attention over the cached
stream positions:

  stage 1  q projection ([l,(h d)] layout) from xT + wq chunks, then
           q rmsnorm (DVE) -> bf16 transposes (PE) -> rope (DVE).
  stage 2  s-loop over all 32 stream tiles. Scores are computed
           TRANSPOSED: scoresT[s,(h l)] = K-tile^T-free @ qT, exp runs
           on ACT directly PSUM->SBUF(bf16); the causal mask reduces to
           a triangle multiply on the final tile only.  PV and the
           denominator ones-matmul accumulate in dedicated PSUM banks.
           Tiles 0..23 come from the kv cache and are ready early;
           tiles 24..31 wait for stage 3.
  stage 3  k/v projections (4 bf16 matmuls per hid-tile) stream under
           stage 2; k rmsnorm uses ones-matmul partition sums, a bf16
           broadcast matmul, and DVE reciprocals of the broadcast;
           norm weights + SCALE are folded into host rope tables.
  stage 4  normalize (reciprocal of denominator, broadcast matmul),
           then the output projection head-outer over 8 PSUM banks,
           streaming partial y chunks back to HBM.

All matmuls are bf16 (fp32 matmuls run at 1/4 rate). ACT only runs exp
and the two sqrt calls, so its Exp table stays loaded through the
s-loop; every cast/copy is DVE.
"""

import os
import sys

sys.path.insert(0, "/opt/trn_rl_repo")

import numpy as np
import ml_dtypes

import concourse.bass as bass
import concourse.bacc as bacc
import concourse.tile as tile
from concourse import mybir
from concourse.bass_utils import run_bass_kernel_spmd

BF16 = ml_dtypes.bfloat16

H, HKV, D, HALF = 32, 8, 128, 64
L, T, S, HID = 128, 1024, 4096, 4096
REP = H // HKV          # q heads per kv head (= per core)
EPS = 1e-6
SCALE = D ** -0.5
NCORES = 8
KT = HID // 128         # 32 contraction tiles for projections
ST = S // 128           # 32 s tiles for attention
SOLD = S - T            # 3072 cached stream positions kept
TNEW = T                # 1024 newly projected stream positions
STOLD = SOLD // 128     # 24 cached s tiles
STNEW = TNEW // 128     # 8 new s tiles

FP32 = mybir.dt.float32
BF16_DT = mybir.dt.bfloat16

_PROGRAM_CACHE = {}

# Filled by kernel() when BASS_KERNEL_TRACE=1; read by test.py.
LAST_RESULTS = None


def _build_program():
    nc = bacc.Bacc("TRN2", target_bir_lowering=False, debug=False,
                   num_devices=NCORES)

    # ---- external I/O (per-core values supplied via in_maps) ----
    cT = nc.declare_dram_parameter("cT", [HID, T], BF16_DT, isOutput=False)
    wkvT = nc.declare_dram_parameter("wkvT", [HID, 256], BF16_DT, isOutput=False)
    wqT = nc.declare_dram_parameter("wqT", [HID, 512], BF16_DT, isOutput=False)
    xTp = nc.declare_dram_parameter("xTp", [128, KT * 128], BF16_DT, isOutput=False)
    woP = nc.declare_dram_parameter("woP", [128, REP, HID // 512, 512], BF16_DT, isOutput=False)
    ktold = nc.declare_dram_parameter("ktold", [D, SOLD], BF16_DT, isOutput=False)
    voldP = nc.declare_dram_parameter("voldP", [128, SOLD], BF16_DT, isOutput=False)
    identb2 = nc.declare_dram_parameter("identb2", [128, 128], BF16_DT, isOutput=False)
    triP = nc.declare_dram_parameter("triP", [128, 512], BF16_DT, isOutput=False)
    cosq = nc.declare_dram_parameter("cosq", [D, L], BF16_DT, isOutput=False)
    sinq = nc.declare_dram_parameter("sinq", [D, L], BF16_DT, isOutput=False)
    cosk = nc.declare_dram_parameter("cosk", [D, TNEW], BF16_DT, isOutput=False)
    sink = nc.declare_dram_parameter("sink", [D, TNEW], BF16_DT, isOutput=False)
    y = nc.declare_dram_parameter("y", [L, HID], FP32, isOutput=True)

    with tile.TileContext(nc) as tc:
        _emit(nc, tc, cT=cT, wkvT=wkvT, wqT=wqT, xTp=xTp, woP=woP,
              ktold=ktold, voldP=voldP, identb2=identb2, triP=triP,
              cosq=cosq, sinq=sinq, cosk=cosk, sink=sink, y=y)
    nc.compile()
    return nc


def _emit(nc, tc, *, cT, wkvT, wqT, xTp, woP, ktold, voldP, identb2, triP,
          cosq, sinq, cosk, sink, y):
    from contextlib import ExitStack

    ctx = ExitStack()
    with ctx:
        # ---------------- pools ----------------
        consts = ctx.enter_context(tc.tile_pool(name="consts", bufs=1))
        streams = ctx.enter_context(tc.tile_pool(name="streams", bufs=1))
        wqp = ctx.enter_context(tc.tile_pool(name="wqp", bufs=3))
        ctp = ctx.enter_context(tc.tile_pool(name="ctp", bufs=10))
        wkvp = ctx.enter_context(tc.tile_pool(name="wkvp", bufs=10))
        wop = ctx.enter_context(tc.tile_pool(name="wop", bufs=4))
        ntmp = ctx.enter_context(tc.tile_pool(name="ntmp", bufs=1))
        sloop = ctx.enter_context(tc.tile_pool(name="sloop", bufs=6))
        ysbp = ctx.enter_context(tc.tile_pool(name="ysbp", bufs=2))
        psA = ctx.enter_context(tc.tile_pool(name="psA", bufs=1, space="PSUM"))
        psS = ctx.enter_context(tc.tile_pool(name="psS", bufs=2, space="PSUM"))

        # ---------------- constants ----------------
        ones_colb = consts.tile([128, 1], BF16_DT, tag="ones_colb")
        nc.vector.memset(ones_colb, 1.0)
        ones_rowb = consts.tile([1, 128], BF16_DT, tag="ones_rowb")
        nc.vector.memset(ones_rowb, 1.0)
        eps_t = consts.tile([128, 1], FP32, tag="eps")
        nc.vector.memset(eps_t, EPS)

        # early resident loads on the qAct ring (ACT is idle until exp)
        identb = consts.tile([128, 128], BF16_DT, tag="identb")
        nc.scalar.dma_start(identb[:], identb2[:])
        cosq_t = consts.tile([D, L], BF16_DT, tag="cosq")
        nc.scalar.dma_start(cosq_t[:], cosq[:])
        sinq_t = consts.tile([D, L], BF16_DT, tag="sinq")
        nc.scalar.dma_start(sinq_t[:], sinq[:])
        kts_old = streams.tile([128, SOLD], BF16_DT, tag="kts_old")
        nc.scalar.dma_start(kts_old[:], ktold[:])
        vt_old = streams.tile([128, SOLD], BF16_DT, tag="vt_old")
        nc.scalar.dma_start(vt_old[:], voldP[:])
        tri_t = consts.tile([128, 512], BF16_DT, tag="tri")
        nc.scalar.dma_start(tri_t[:], triP[:])
        cosk_t = consts.tile([D, TNEW], BF16_DT, tag="cosk")
        nc.scalar.dma_start(cosk_t[:], cosk[:])
        sink_t = consts.tile([D, TNEW], BF16_DT, tag="sink")
        nc.scalar.dma_start(sink_t[:], sink[:])

        # ---------------- stage 1: q path ----------------
        xT_res = streams.tile([128, KT * 128], BF16_DT, tag="xT")
        nc.sync.dma_start(xT_res[:], xTp[:])

        ps_q = psA.tile([128, 512], FP32, tag="A")
        with nc.named_scope("qproj"):
            for c in range(8):          # 8 wq chunks of 4 hid-tiles each
                wq_c = wqp.tile([128, 4, 512], BF16_DT, tag="wq")
                nc.sync.dma_start(
                    wq_c[:],
                    wqT[c * 512:(c + 1) * 512, :].rearrange(
                        "(j p) e -> p j e", p=128))
                for j in range(4):
                    k = c * 4 + j
                    nc.tensor.matmul(ps_q[:],
                                     xT_res[:, k * 128:(k + 1) * 128],
                                     wq_c[:, j, :],
                                     start=(k == 0), stop=(k == KT - 1))

        with nc.named_scope("qnorm"):
            qsb = ntmp.tile([128, 512], FP32, tag="qsb")
            nc.vector.tensor_copy(qsb[:], ps_q[:])
            qsq = ntmp.tile([128, 512], FP32, tag="qsq")
            nc.vector.tensor_mul(qsq[:], qsb[:], qsb[:])
            qsos = ntmp.tile([128, REP], FP32, tag="qsos")
            nc.vector.reduce_sum(
                qsos[:],
                qsq[:].rearrange("p (h d) -> p h d", h=REP),
                axis=mybir.AxisListType.X,
            )
            qrstd = ntmp.tile([128, REP], FP32, tag="qrstd")
            nc.scalar.activation(qrstd[:], qsos[:],
                                 mybir.ActivationFunctionType.Sqrt,
                                 bias=eps_t[:], scale=1.0 / D)
            nc.vector.reciprocal(qrstd[:], qrstd[:])
            qn = ntmp.tile([128, 512], BF16_DT, tag="qn")
            for h in range(REP):
                nc.vector.tensor_scalar_mul(qn[:, h * 128:(h + 1) * 128],
                                            qsb[:, h * 128:(h + 1) * 128],
                                            qrstd[:, h:h + 1])
            # bf16 PE transposes -> qT layout [d, (h l)]
            qtw = ntmp.tile([128, 512], BF16_DT, tag="qtw")
            for h in range(REP):
                ps_qT = psS.tile([128, 128], BF16_DT, tag="sc")
                nc.tensor.transpose(ps_qT[:], qn[:, h * 128:(h + 1) * 128],
                                    identb[:])
                nc.vector.tensor_copy(qtw[:, h * 128:(h + 1) * 128], ps_qT[:])
            # rope (sign + SCALE + q_norm_w folded into host tables)
            qrot = ntmp.tile([128, 512], BF16_DT, tag="qrot")
            nc.scalar.dma_start(qrot[0:HALF, :], qtw[HALF:D, :])
            nc.scalar.dma_start(qrot[HALF:D, :], qtw[0:HALF, :])
            qa = ntmp.tile([128, 512], BF16_DT, tag="qa")
            qb = ntmp.tile([128, 512], BF16_DT, tag="qb")
            for h in range(REP):
                sl = slice(h * 128, (h + 1) * 128)
                nc.vector.tensor_mul(qa[:, sl], qtw[:, sl], cosq_t[:])
                nc.vector.tensor_mul(qb[:, sl], qrot[:, sl], sinq_t[:])
            qT_all = streams.tile([128, 512], BF16_DT, tag="qT_all")
            nc.vector.tensor_add(qT_all[:], qa[:], qb[:])

        # ---------------- stage 2: attention s-loop ----------------
        kts_new = streams.tile([128, TNEW], BF16_DT, tag="kts_new")
        vt_new = streams.tile([128, TNEW], BF16_DT, tag="vt_new")
        ps_sum = psA.tile([1, 512], FP32, tag="A")
        ps_o = psA.tile([128, 512], FP32, tag="B")

        def s_iter(s, ksrc, vsrc):
            ps_sc = psS.tile([128, 512], FP32, tag="sc")
            nc.tensor.matmul(ps_sc[:], ksrc, qT_all[:])
            ex = sloop.tile([128, 512], BF16_DT, tag="ex")
            nc.scalar.activation(ex[:], ps_sc[:],
                                 mybir.ActivationFunctionType.Exp)
            if s == ST - 1:
                exm = ntmp.tile([128, 512], BF16_DT, tag="exm")
                nc.vector.tensor_mul(exm[:], ex[:], tri_t[:])
                ex = exm
            nc.tensor.matmul(ps_sum[:], ones_colb[:], ex[:],
                             start=(s == 0), stop=(s == ST - 1))
            nc.tensor.matmul(ps_o[:], vsrc, ex[:],
                             start=(s == 0), stop=(s == ST - 1))

        with nc.named_scope("sloop_old"):
            for s in range(STOLD):
                s_iter(s, kts_old[:, s * 128:(s + 1) * 128],
                       vt_old[:, s * 128:(s + 1) * 128])

        # ---------------- stage 3: k/v projections + norm ----------------
        ps_k0 = psA.tile([128, 512], FP32, tag="E")
        ps_k1 = psA.tile([128, 512], FP32, tag="F")
        ps_v0 = psA.tile([128, 512], FP32, tag="G")
        ps_v1 = psA.tile([128, 512], FP32, tag="H")
        with nc.named_scope("kvproj"):
            for k in range(KT):
                ct_k = ctp.tile([128, T], BF16_DT, tag="ct")
                nc.sync.dma_start(ct_k[:], cT[k * 128:(k + 1) * 128, :])
                w_k = wkvp.tile([128, 256], BF16_DT, tag="wkv")
                nc.sync.dma_start(w_k[:], wkvT[k * 128:(k + 1) * 128, :])
                st = (k == 0)
                sp = (k == KT - 1)
                nc.tensor.matmul(ps_k0[:], w_k[:, 0:128], ct_k[:, 0:512],
                                 start=st, stop=sp)
                nc.tensor.matmul(ps_k1[:], w_k[:, 0:128], ct_k[:, 512:1024],
                                 start=st, stop=sp)
                nc.tensor.matmul(ps_v0[:], w_k[:, 128:256], ct_k[:, 0:512],
                                 start=st, stop=sp)
                nc.tensor.matmul(ps_v1[:], w_k[:, 128:256], ct_k[:, 512:1024],
                                 start=st, stop=sp)

        # wo loads: on the qSP ring AFTER the cT stream (FIFO keeps them
        # from stealing HBM bandwidth from the PE-feeding loads)
        wo_h = []
        for h in range(REP):
            wt = wop.tile([128, HID // 512, 512], BF16_DT, tag="wo")
            nc.sync.dma_start(wt[:], woP[:, h, :, :])
            wo_h.append(wt)

        with nc.named_scope("knorm"):
            kc = ntmp.tile([128, TNEW], BF16_DT, tag="kc")
            nc.vector.tensor_copy(kc[:, 0:512], ps_k0[:])
            nc.vector.tensor_copy(kc[:, 512:1024], ps_k1[:])
            vsb = ntmp.tile([128, TNEW], BF16_DT, tag="vsb")
            nc.vector.tensor_copy(vsb[:, 0:512], ps_v0[:])
            nc.vector.tensor_copy(vsb[:, 512:1024], ps_v1[:])
            ksq = ntmp.tile([128, TNEW], BF16_DT, tag="ksq")
            nc.vector.tensor_mul(ksq[:, 0:512], kc[:, 0:512], kc[:, 0:512])
            nc.vector.tensor_mul(ksq[:, 512:1024], kc[:, 512:1024],
                                 kc[:, 512:1024])
            ps_sos0 = psA.tile([1, 512], FP32, tag="E")
            ps_sos1 = psA.tile([1, 512], FP32, tag="F")
            nc.tensor.matmul(ps_sos0[:], ones_colb[:], ksq[:, 0:512])
            nc.tensor.matmul(ps_sos1[:], ones_colb[:], ksq[:, 512:1024])
            kstd = ntmp.tile([1, TNEW], BF16_DT, tag="kstd")
            nc.scalar.activation(kstd[:, 0:512], ps_sos0[:],
                                 mybir.ActivationFunctionType.Sqrt,
                                 bias=eps_t[0:1, :], scale=1.0 / D)
            nc.scalar.activation(kstd[:, 512:1024], ps_sos1[:],
                                 mybir.ActivationFunctionType.Sqrt,
                                 bias=eps_t[0:1, :], scale=1.0 / D)
            # broadcast std across partitions, then DVE reciprocal
            ps_kb0 = psA.tile([128, 512], FP32, tag="E")
            ps_kb1 = psA.tile([128, 512], FP32, tag="F")
            nc.tensor.matmul(ps_kb0[:], ones_rowb[:], kstd[:, 0:512])
            nc.tensor.matmul(ps_kb1[:], ones_rowb[:], kstd[:, 512:1024])
            krr = ntmp.tile([128, TNEW], BF16_DT, tag="krr")
            with nc.allow_low_precision(reason="bf16 rstd, not an accum"):
                nc.vector.reciprocal(krr[:, 0:512], ps_kb0[:])
                nc.vector.reciprocal(krr[:, 512:1024], ps_kb1[:])
            knw = ntmp.tile([128, TNEW], BF16_DT, tag="knw")
            nc.vector.tensor_mul(knw[:], kc[:], krr[:])
            # rope (sign + k_norm_w folded into host tables)
            krot = ntmp.tile([128, TNEW], BF16_DT, tag="krot")
            nc.scalar.dma_start(krot[0:HALF, :], knw[HALF:D, :])
            nc.scalar.dma_start(krot[HALF:D, :], knw[0:HALF, :])
            ka = ntmp.tile([128, TNEW], BF16_DT, tag="ka")
            nc.vector.tensor_mul(ka[:], knw[:], cosk_t[:])
            kb = ntmp.tile([128, TNEW], BF16_DT, tag="kb")
            nc.vector.tensor_mul(kb[:], krot[:], sink_t[:])
            nc.vector.tensor_add(kts_new[:], ka[:], kb[:])
            # v transposes into stream layout [s_local, d]
            for i in range(STNEW):
                ps_vT = psA.tile([128, 128], BF16_DT,
                                 tag=("G" if i % 2 == 0 else "H"))
                nc.tensor.transpose(ps_vT[:], vsb[:, i * 128:(i + 1) * 128],
                                    identb[:])
                nc.vector.tensor_copy(vt_new[:, i * 128:(i + 1) * 128],
                                      ps_vT[:])

        # s-loop over the newly projected tiles (emitted after the
        # kts_new / vt_new writes so Tile sees the RAW dependency)
        with nc.named_scope("sloop_new"):
            for s in range(STOLD, ST):
                j = s - STOLD
                s_iter(s, kts_new[:, j * 128:(j + 1) * 128],
                       vt_new[:, j * 128:(j + 1) * 128])

        # ---------------- stage 4: normalize + output projection --------
        with nc.named_scope("fin"):
            rec = ntmp.tile([1, 512], BF16_DT, tag="rec")
            with nc.allow_low_precision(reason="bf16 softmax denom recip"):
                nc.vector.reciprocal(rec[:], ps_sum[:])
            ps_rb = psS.tile([128, 512], FP32, tag="sc")
            nc.tensor.matmul(ps_rb[:], ones_rowb[:], rec[:])
            osb = ntmp.tile([128, 512], FP32, tag="osb")
            nc.vector.tensor_copy(osb[:], ps_o[:])
            attT = streams.tile([128, 512], BF16_DT, tag="attT")
            nc.vector.tensor_mul(attT[:], osb[:], ps_rb[:])

        with nc.named_scope("oproj"):
            tags = ["A", "B", "E", "F", "G", "H"]
            ps_y = []
            for e in range(HID // 512):
                if e < 6:
                    ps_y.append(psA.tile([128, 512], FP32, tag=tags[e],
                                         name=f"ps_y{e}"))
                else:
                    ps_y.append(psS.tile([128, 512], FP32, tag="sc",
                                         name=f"ps_y{e}"))
            for h in range(REP):
                for e in range(HID // 512):
                    nc.tensor.matmul(
                        ps_y[e][:], attT[:, h * 128:(h + 1) * 128],
                        wo_h[h][:, e, :],
                        start=(h == 0), stop=(h == REP - 1))
            for e in range(HID // 512):
                ysb = ysbp.tile([128, 512], FP32, tag="ysb")
                if e % 2 == 0:
                    nc.vector.tensor_copy(ysb[:], ps_y[e][:])
                else:
                    nc.scalar.copy(ysb[:], ps_y[e][:])
                nc.sync.dma_start(y[:, e * 512:(e + 1) * 512], ysb[:])


def _prepare_inputs(x, x_ctx, cos_q, sin_q, cos_k, sin_k, kv_cache,
                    causal_mask, Wq, Wk, Wv, Wo, q_norm_w, k_norm_w):
    """Host-side sharding/preprocessing. Returns list of per-core in_maps."""
    f32 = np.float32
    x = np.asarray(x, f32)
    x_ctx = np.asarray(x_ctx, f32)
    c = np.concatenate([x_ctx[0], x[0]], axis=0)          # [T, HID]
    cT = np.ascontiguousarray(c.T).astype(BF16)           # [HID, T]

    # x.T packed [p, (k 128l)]: xTp[p, k*128+l] = c.T[k*128+p, T-L+l]
    xTp = np.ascontiguousarray(
        c.T[:, T - L:T].reshape(KT, 128, L).transpose(1, 0, 2)
        .reshape(128, KT * L)).astype(BF16)

    # final-tile multiplicative mask: allowed iff s_local <= l,
    # replicated across the 4 q heads -> [s_local, (h l)]
    tri = (np.arange(128)[:, None] <= np.arange(128)[None, :]).astype(f32)
    triP = np.ascontiguousarray(np.tile(tri, (1, REP))).astype(BF16)

    qw = np.asarray(q_norm_w, f32).reshape(D)
    kw = np.asarray(k_norm_w, f32).reshape(D)
    rot_src = (np.arange(D) + HALF) % D                   # rotate-half source

    cosqT = np.asarray(cos_q, f32)[0, 0].T * SCALE * qw[:, None]
    sinqT = np.asarray(sin_q, f32)[0, 0].T.copy()
    sinqT[:HALF] = -sinqT[:HALF]
    sinqT = sinqT * SCALE * qw[rot_src][:, None]
    coskT = np.asarray(cos_k, f32)[0, 0].T * kw[:, None]
    sinkT = np.asarray(sin_k, f32)[0, 0].T.copy()
    sinkT[:HALF] = -sinkT[:HALF]
    sinkT = sinkT * kw[rot_src][:, None]

    Wq = np.asarray(Wq, f32)
    Wk = np.asarray(Wk, f32)
    Wv = np.asarray(Wv, f32)
    Wo = np.asarray(Wo, f32)
    kv = np.asarray(kv_cache, f32)

    in_maps = []
    for cidx in range(NCORES):
        hd = slice(cidx * REP * D, (cidx + 1) * REP * D)
        wq_c = Wq[hd].reshape(REP, D, HID)
        wq_c = wq_c - wq_c.mean(axis=1, keepdims=True)    # fold mean-subtract
        wq_c = wq_c.reshape(REP * D, HID)
        wk_c = Wk[cidx * D:(cidx + 1) * D]
        wk_c = wk_c - wk_c.mean(axis=0, keepdims=True)
        wv_c = Wv[cidx * D:(cidx + 1) * D]
        wkvT = np.concatenate([wk_c.T, wv_c.T], axis=1)   # [HID, 256]
        wqTc = np.ascontiguousarray(wq_c.T)               # [HID, 512]
        # wo packed [p, h, e_chunk, 512]: woP[p,h,j,e'] = Wo.T[h*128+p, j*512+e']
        woTc = Wo[:, hd].T.reshape(REP, 128, HID // 512, 512)
        woP = np.ascontiguousarray(woTc.transpose(1, 0, 2, 3))
        ktold = np.ascontiguousarray(kv[0, cidx, T:, :].T)  # [D, SOLD]
        # vold packed [s_local, (tile d)]: voldP[p, n*128+d] = v[n*128+p, d]
        voldP = np.ascontiguousarray(
            kv[1, cidx, T:, :].reshape(SOLD // 128, 128, D)
            .transpose(1, 0, 2).reshape(128, SOLD))
        in_maps.append(dict(
            cT=cT,
            wkvT=np.ascontiguousarray(wkvT).astype(BF16),
            wqT=wqTc.astype(BF16),
            xTp=xTp,
            woP=woP.astype(BF16),
            ktold=ktold.astype(BF16),
            voldP=voldP.astype(BF16),
            identb2=np.eye(128, dtype=f32).astype(BF16),
            triP=triP,
            cosq=cosqT.astype(BF16), sinq=sinqT.astype(BF16),
            cosk=coskT.astype(BF16), sink=sinkT.astype(BF16),
        ))
    return in_maps


def kernel(**inputs) -> np.ndarray:
    global LAST_RESULTS
    if "nc" not in _PROGRAM_CACHE:
        _PROGRAM_CACHE["nc"] = _build_program()
    nc = _PROGRAM_CACHE["nc"]
    in_maps = _prepare_inputs(**inputs)
    trace = bool(int(os.environ.get("BASS_KERNEL_TRACE", "0")))
    res = run_bass_kernel_spmd(nc, in_maps, list(range(NCORES)), trace=trace)
    LAST_RESULTS = res
    y = np.zeros((L, HID), np.float64)
    for cidx in range(NCORES):
        y += res.results[cidx]["y"].astype(np.float64)
    return y.astype(np.float32).reshape(1, L, HID)


# revision 12
# speedup vs baseline: 1.2724x; 1.0906x over previous
"""Trainium2 Bass kernel for nn_DFlashAttentionSlide (GQA attention block).

Sharding: tensor-parallel over heads across 8 NeuronCores. Core c owns
kv head c and q heads [4c, 4c+4). Activations (x/x_ctx) are replicated;
weights / kv-cache are sharded along the head dim; the output projection
is contraction-sharded, so each core returns a partial [L, HID] output
that the host sums.

Device-side schedule (per core). The PE instruction count is the hard
floor (~300 bf16 matmuls, mostly N=512), so the schedule is built to
keep them streaming back-to-back:

  stage 1  q projection from xT + wq chunks, q rmsnorm (DVE) -> bf16
           transposes (PE) -> rope (DVE).  DMA-paced (~5 MB).
  stage 2  s-loop over the 24 cached stream tiles.  ScoresT[s,(h l)]
           via K-tile stationary / qT moving; exp on ACT directly
           PSUM->SBUF bf16 (4 rotating PSUM banks of lookahead); the
           causal mask is a triangle multiply on the final tile only.
           PV and the denominator ones-matmul accumulate in dedicated
           banks.  Runs while cT streams into SBUF (resident).
  stage 3  K-pass: k projection only (2 PSUM banks), then k-norm +
           rope (hidden under the V-pass); V-pass re-reads the
           RESIDENT cT (no extra HBM traffic), then v transposes.
  stage 4  s-loop over the 8 new tiles, softmax normalize (fast
           approx reciprocal), output projection head-outer across all
           8 PSUM banks, y chunks streamed out on the second ring.

All matmuls are bf16 (fp32 runs at 1/4 rate).  ACT runs only exp and
the two sqrts; casts/copies are DVE so the Exp table stays loaded.
DMA: big (~1MB) transfers, two HWDGE rings ordered by need-time.
"""

import os
import sys

sys.path.insert(0, "/opt/trn_rl_repo")

import numpy as np
import ml_dtypes

import concourse.bass as bass
import concourse.bacc as bacc
import concourse.tile as tile
from concourse import mybir
from concourse.bass_utils import run_bass_kernel_spmd

BF16 = ml_dtypes.bfloat16

H, HKV, D, HALF = 32, 8, 128, 64
L, T, S, HID = 128, 1024, 4096, 4096
REP = H // HKV          # q heads per kv head (= per core)
EPS = 1e-6
SCALE = D ** -0.5
NCORES = 8
KT = HID // 128         # 32 contraction tiles for projections
ST = S // 128           # 32 s tiles for attention
SOLD = S - T            # 3072 cached stream positions kept
TNEW = T                # 1024 newly projected stream positions
STOLD = SOLD // 128     # 24 cached s tiles
STNEW = TNEW // 128     # 8 new s tiles

FP32 = mybir.dt.float32
BF16_DT = mybir.dt.bfloat16

_PROGRAM_CACHE = {}

# Filled by kernel() when BASS_KERNEL_TRACE=1; read by test.py.
LAST_RESULTS = None


def _build_program():
    nc = bacc.Bacc("TRN2", target_bir_lowering=False, debug=False,
                   num_devices=NCORES)

    # ---- external I/O (per-core values supplied via in_maps) ----
    cT = nc.declare_dram_parameter("cT", [HID, T], BF16_DT, isOutput=False)
    wkvT = nc.declare_dram_parameter("wkvT", [HID, 256], BF16_DT, isOutput=False)
    wqT = nc.declare_dram_parameter("wqT", [HID, 512], BF16_DT, isOutput=False)
    xTp = nc.declare_dram_parameter("xTp", [128, KT * 128], BF16_DT, isOutput=False)
    woP = nc.declare_dram_parameter("woP", [128, REP, HID // 512, 512], BF16_DT, isOutput=False)
    ktold = nc.declare_dram_parameter("ktold", [D, SOLD], BF16_DT, isOutput=False)
    voldP = nc.declare_dram_parameter("voldP", [128, SOLD], BF16_DT, isOutput=False)
    identb2 = nc.declare_dram_parameter("identb2", [128, 128], BF16_DT, isOutput=False)
    triP = nc.declare_dram_parameter("triP", [128, 512], BF16_DT, isOutput=False)
    cosq = nc.declare_dram_parameter("cosq", [D, L], BF16_DT, isOutput=False)
    sinq = nc.declare_dram_parameter("sinq", [D, L], BF16_DT, isOutput=False)
    cosk = nc.declare_dram_parameter("cosk", [D, TNEW], BF16_DT, isOutput=False)
    sink = nc.declare_dram_parameter("sink", [D, TNEW], BF16_DT, isOutput=False)
    y = nc.declare_dram_parameter("y", [L, HID], FP32, isOutput=True)

    with tile.TileContext(nc) as tc:
        _emit(nc, tc, cT=cT, wkvT=wkvT, wqT=wqT, xTp=xTp, woP=woP,
              ktold=ktold, voldP=voldP, identb2=identb2, triP=triP,
              cosq=cosq, sinq=sinq, cosk=cosk, sink=sink, y=y)
    nc.compile()
    return nc


def _emit(nc, tc, *, cT, wkvT, wqT, xTp, woP, ktold, voldP, identb2, triP,
          cosq, sinq, cosk, sink, y):
    from contextlib import ExitStack

    ctx = ExitStack()
    with ctx:
        # ---------------- pools ----------------
        consts = ctx.enter_context(tc.tile_pool(name="consts", bufs=1))
        streams = ctx.enter_context(tc.tile_pool(name="streams", bufs=1))
        wqp = ctx.enter_context(tc.tile_pool(name="wqp", bufs=2))
        wop = ctx.enter_context(tc.tile_pool(name="wop", bufs=4))
        ntmp = ctx.enter_context(tc.tile_pool(name="ntmp", bufs=1))
        sloop = ctx.enter_context(tc.tile_pool(name="sloop", bufs=6))
        ysbp = ctx.enter_context(tc.tile_pool(name="ysbp", bufs=2))
        psA = ctx.enter_context(tc.tile_pool(name="psA", bufs=1, space="PSUM"))
        psS = ctx.enter_context(tc.tile_pool(name="psS", bufs=4, space="PSUM"))

        # ---------------- constants ----------------
        ones_colb = consts.tile([128, 1], BF16_DT, tag="ones_colb")
        nc.vector.memset(ones_colb, 1.0)
        ones_rowb = consts.tile([1, 128], BF16_DT, tag="ones_rowb")
        nc.vector.memset(ones_rowb, 1.0)
        eps_t = consts.tile([128, 1], FP32, tag="eps")
        nc.vector.memset(eps_t, EPS)

        # small early residents on the qAct ring (ACT is idle until exp)
        identb = consts.tile([128, 128], BF16_DT, tag="identb")
        nc.scalar.dma_start(identb[:], identb2[:])
        cosq_t = consts.tile([D, L], BF16_DT, tag="cosq")
        nc.scalar.dma_start(cosq_t[:], cosq[:])
        sinq_t = consts.tile([D, L], BF16_DT, tag="sinq")
        nc.scalar.dma_start(sinq_t[:], sinq[:])
        tri_t = consts.tile([128, 512], BF16_DT, tag="tri")
        nc.scalar.dma_start(tri_t[:], triP[:])
        cosk_t = consts.tile([D, TNEW], BF16_DT, tag="cosk")
        nc.scalar.dma_start(cosk_t[:], cosk[:])
        sink_t = consts.tile([D, TNEW], BF16_DT, tag="sink")
        nc.scalar.dma_start(sink_t[:], sink[:])

        # ---------------- stage 1: q path (qSP ring) ----------------
        xT_res = streams.tile([128, KT * 128], BF16_DT, tag="xT")
        nc.sync.dma_start(xT_res[:], xTp[:])

        ps_q = psA.tile([128, 512], FP32, tag="A")
        with nc.named_scope("qproj"):
            for c in range(4):          # 4 wq chunks of 8 hid-tiles each
                wq_c = wqp.tile([128, 8, 512], BF16_DT, tag="wq")
                nc.sync.dma_start(
                    wq_c[:],
                    wqT[c * 1024:(c + 1) * 1024, :].rearrange(
                        "(j p) e -> p j e", p=128))
                for j in range(8):
                    k = c * 8 + j
                    nc.tensor.matmul(ps_q[:],
                                     xT_res[:, k * 128:(k + 1) * 128],
                                     wq_c[:, j, :],
                                     start=(k == 0), stop=(k == KT - 1))

        # cached k/v stream tiles, split in halves so attention can start
        # after the first half lands
        kts_oa = streams.tile([128, SOLD // 2], BF16_DT, tag="kts_oa")
        nc.sync.dma_start(kts_oa[:], ktold[:, 0:SOLD // 2])
        vt_oa = streams.tile([128, SOLD // 2], BF16_DT, tag="vt_oa")
        nc.sync.dma_start(vt_oa[:], voldP[:, 0:SOLD // 2])
        kts_ob = streams.tile([128, SOLD // 2], BF16_DT, tag="kts_ob")
        nc.sync.dma_start(kts_ob[:], ktold[:, SOLD // 2:SOLD])
        vt_ob = streams.tile([128, SOLD // 2], BF16_DT, tag="vt_ob")
        nc.sync.dma_start(vt_ob[:], voldP[:, SOLD // 2:SOLD])

        with nc.named_scope("qnorm"):
            qsb = ntmp.tile([128, 512], FP32, tag="qsb")
            nc.vector.tensor_copy(qsb[:], ps_q[:])
            qsq = ntmp.tile([128, 512], FP32, tag="qsq")
            nc.vector.tensor_mul(qsq[:], qsb[:], qsb[:])
            qsos = ntmp.tile([128, REP], FP32, tag="qsos")
            nc.vector.reduce_sum(
                qsos[:],
                qsq[:].rearrange("p (h d) -> p h d", h=REP),
                axis=mybir.AxisListType.X,
            )
            qstd = ntmp.tile([128, REP], FP32, tag="qstd")
            nc.scalar.activation(qstd[:], qsos[:],
                                 mybir.ActivationFunctionType.Sqrt,
                                 bias=eps_t[:], scale=1.0 / D)
            qrstd = ntmp.tile([128, REP], FP32, tag="qrstd")
            nc.vector.reciprocal_approx_fast(out=qrstd[:], in_=qstd[:])
            qn = ntmp.tile([128, 512], BF16_DT, tag="qn")
            for h in range(REP):
                nc.vector.tensor_scalar_mul(qn[:, h * 128:(h + 1) * 128],
                                            qsb[:, h * 128:(h + 1) * 128],
                                            qrstd[:, h:h + 1])
            # bf16 PE transposes -> qT layout [d, (h l)]
            qtw = ntmp.tile([128, 512], BF16_DT, tag="qtw")
            for h in range(REP):
                ps_qT = psS.tile([128, 128], BF16_DT, tag="sc")
                nc.tensor.transpose(ps_qT[:], qn[:, h * 128:(h + 1) * 128],
                                    identb[:])
                nc.vector.tensor_copy(qtw[:, h * 128:(h + 1) * 128], ps_qT[:])
            # rope (sign + SCALE + q_norm_w folded into host tables)
            qrot = ntmp.tile([128, 512], BF16_DT, tag="qrot")
            nc.scalar.dma_start(qrot[0:HALF, :], qtw[HALF:D, :])
            nc.scalar.dma_start(qrot[HALF:D, :], qtw[0:HALF, :])
            qa = ntmp.tile([128, 512], BF16_DT, tag="qa")
            qb = ntmp.tile([128, 512], BF16_DT, tag="qb")
            for h in range(REP):
                sl = slice(h * 128, (h + 1) * 128)
                nc.vector.tensor_mul(qa[:, sl], qtw[:, sl], cosq_t[:])
                nc.vector.tensor_mul(qb[:, sl], qrot[:, sl], sinq_t[:])
            qT_all = streams.tile([128, 512], BF16_DT, tag="qT_all")
            nc.vector.tensor_add(qT_all[:], qa[:], qb[:])

        # ---------------- stage 2: attention s-loop ----------------
        kts_new = streams.tile([128, TNEW], BF16_DT, tag="kts_new")
        vt_new = streams.tile([128, TNEW], BF16_DT, tag="vt_new")
        ps_sum = psA.tile([1, 512], FP32, tag="B")
        ps_o = psA.tile([128, 512], FP32, tag="A")

        def s_iter(s, ksrc, vsrc):
            ps_sc = psS.tile([128, 512], FP32, tag="sc")
            nc.tensor.matmul(ps_sc[:], ksrc, qT_all[:])
            ex = sloop.tile([128, 512], BF16_DT, tag="ex")
            nc.scalar.activation(ex[:], ps_sc[:],
                                 mybir.ActivationFunctionType.Exp)
            if s == ST - 1:
                exm = ntmp.tile([128, 512], BF16_DT, tag="exm")
                nc.vector.tensor_mul(exm[:], ex[:], tri_t[:])
                ex = exm
            nc.tensor.matmul(ps_sum[:], ones_colb[:], ex[:],
                             start=(s == 0), stop=(s == ST - 1))
            nc.tensor.matmul(ps_o[:], vsrc, ex[:],
                             start=(s == 0), stop=(s == ST - 1))

        HT = STOLD // 2
        with nc.named_scope("sloop_old"):
            for s in range(STOLD):
                if s < HT:
                    s_iter(s, kts_oa[:, s * 128:(s + 1) * 128],
                           vt_oa[:, s * 128:(s + 1) * 128])
                else:
                    j = s - HT
                    s_iter(s, kts_ob[:, j * 128:(j + 1) * 128],
                           vt_ob[:, j * 128:(j + 1) * 128])

        # ---------------- stage 3: projections from resident cT ---------
        wkv_res = []
        for c2 in range(2):
            wt = streams.tile([128, 16, 256], BF16_DT, name=f"wkv{c2}",
                              tag=f"wkv{c2}")
            nc.sync.dma_start(
                wt[:],
                wkvT[c2 * 2048:(c2 + 1) * 2048, :].rearrange(
                    "(j p) n -> p j n", p=128))
            wkv_res.append(wt)
        ctg = []
        for g in range(8):
            ct_t = streams.tile([128, 4, T], BF16_DT, name=f"ct{g}",
                                tag=f"ct{g}")
            nc.sync.dma_start(
                ct_t[:],
                cT[g * 512:(g + 1) * 512, :].rearrange(
                    "(j p) t -> p j t", p=128))
            ctg.append(ct_t)
        # wo on the qSP ring behind cT (FIFO -> no bandwidth steal)
        wo_h = []
        for h in range(REP):
            wt = wop.tile([128, HID // 512, 512], BF16_DT, name=f"wo{h}",
                          tag="wo")
            nc.sync.dma_start(wt[:], woP[:, h, :, :])
            wo_h.append(wt)

        ps_k0 = psA.tile([128, 512], FP32, tag="C")
        ps_k1 = psA.tile([128, 512], FP32, tag="D")
        with nc.named_scope("kpass"):
            for k in range(KT):
                wk = wkv_res[k // 16][:, k % 16, 0:128]
                ct_sl = ctg[k // 4][:, k % 4, :]
                st = (k == 0)
                sp = (k == KT - 1)
                nc.tensor.matmul(ps_k0[:], wk, ct_sl[:, 0:512],
                                 start=st, stop=sp)
                nc.tensor.matmul(ps_k1[:], wk, ct_sl[:, 512:1024],
                                 start=st, stop=sp)

        with nc.named_scope("knorm"):
            kc = ntmp.tile([128, TNEW], BF16_DT, tag="qsb")
            nc.vector.tensor_copy(kc[:, 0:512], ps_k0[:])
            nc.vector.tensor_copy(kc[:, 512:1024], ps_k1[:])
            ksq = ntmp.tile([128, TNEW], BF16_DT, tag="qsq")
            nc.vector.tensor_mul(ksq[:, 0:512], kc[:, 0:512], kc[:, 0:512])
            nc.vector.tensor_mul(ksq[:, 512:1024], kc[:, 512:1024],
                                 kc[:, 512:1024])
            ps_sos0 = psS.tile([1, 512], FP32, tag="sc")
            ps_sos1 = psS.tile([1, 512], FP32, tag="sc")
            nc.tensor.matmul(ps_sos0[:], ones_colb[:], ksq[:, 0:512])
            nc.tensor.matmul(ps_sos1[:], ones_colb[:], ksq[:, 512:1024])
            kstd = ntmp.tile([1, TNEW], BF16_DT, tag="kstd")
            nc.scalar.activation(kstd[:, 0:512], ps_sos0[:],
                                 mybir.ActivationFunctionType.Sqrt,
                                 bias=eps_t[0:1, :], scale=1.0 / D)
            nc.scalar.activation(kstd[:, 512:1024], ps_sos1[:],
                                 mybir.ActivationFunctionType.Sqrt,
                                 bias=eps_t[0:1, :], scale=1.0 / D)
            # broadcast std across partitions, then fast DVE reciprocal
            ps_kb0 = psS.tile([128, 512], FP32, tag="sc")
            ps_kb1 = psS.tile([128, 512], FP32, tag="sc")
            nc.tensor.matmul(ps_kb0[:], ones_rowb[:], kstd[:, 0:512])
            nc.tensor.matmul(ps_kb1[:], ones_rowb[:], kstd[:, 512:1024])
            krr = ntmp.tile([128, TNEW], FP32, tag="krr")
            nc.vector.reciprocal_approx_fast(out=krr[:, 0:512], in_=ps_kb0[:])
            nc.vector.reciprocal_approx_fast(out=krr[:, 512:1024],
                                             in_=ps_kb1[:])
            krrb = ntmp.tile([128, TNEW], BF16_DT, tag="qtw")
            nc.vector.tensor_copy(krrb[:], krr[:])
            knw = ntmp.tile([128, TNEW], BF16_DT, tag="qn")
            nc.vector.tensor_mul(knw[:], kc[:], krrb[:])
            # rope (sign + k_norm_w folded into host tables)
            krot = ntmp.tile([128, TNEW], BF16_DT, tag="qrot")
            nc.scalar.dma_start(krot[0:HALF, :], knw[HALF:D, :])
            nc.scalar.dma_start(krot[HALF:D, :], knw[0:HALF, :])
            ka = ntmp.tile([128, TNEW], BF16_DT, tag="qa")
            nc.vector.tensor_mul(ka[:], knw[:], cosk_t[:])
            kb = ntmp.tile([128, TNEW], BF16_DT, tag="qb")
            nc.vector.tensor_mul(kb[:], krot[:], sink_t[:])
            nc.vector.tensor_add(kts_new[:], ka[:], kb[:])

        # V-pass re-reads resident cT; banks C/D freed by the kc copies
        ps_v0 = psA.tile([128, 512], FP32, tag="C")
        ps_v1 = psA.tile([128, 512], FP32, tag="D")
        with nc.named_scope("vpass"):
            for k in range(KT):
                wv = wkv_res[k // 16][:, k % 16, 128:256]
                ct_sl = ctg[k // 4][:, k % 4, :]
                st = (k == 0)
                sp = (k == KT - 1)
                nc.tensor.matmul(ps_v0[:], wv, ct_sl[:, 0:512],
                                 start=st, stop=sp)
                nc.tensor.matmul(ps_v1[:], wv, ct_sl[:, 512:1024],
                                 start=st, stop=sp)
            vsb = ntmp.tile([128, TNEW], BF16_DT, tag="vsb")
            nc.vector.tensor_copy(vsb[:, 0:512], ps_v0[:])
            nc.vector.tensor_copy(vsb[:, 512:1024], ps_v1[:])
            for i in range(STNEW):
                ps_vT = psS.tile([128, 128], BF16_DT, tag="sc",
                                 name=f"ps_vT{i}")
                nc.tensor.transpose(ps_vT[:], vsb[:, i * 128:(i + 1) * 128],
                                    identb[:])
                nc.vector.tensor_copy(vt_new[:, i * 128:(i + 1) * 128],
                                      ps_vT[:])

        # s-loop over the newly projected tiles
        with nc.named_scope("sloop_new"):
            for s in range(STOLD, ST):
                j = s - STOLD
                s_iter(s, kts_new[:, j * 128:(j + 1) * 128],
                       vt_new[:, j * 128:(j + 1) * 128])

        # ---------------- stage 4: normalize + output projection --------
        with nc.named_scope("fin"):
            rec = ntmp.tile([1, 512], FP32, tag="rec")
            nc.vector.reciprocal_approx_fast(out=rec[:], in_=ps_sum[:])
            recb = ntmp.tile([1, 512], BF16_DT, tag="recb")
            nc.vector.tensor_copy(recb[:], rec[:])
            ps_rb = psS.tile([128, 512], FP32, tag="sc")
            nc.tensor.matmul(ps_rb[:], ones_rowb[:], recb[:])
            osb = ntmp.tile([128, 512], FP32, tag="osb")
            nc.vector.tensor_copy(osb[:], ps_o[:])
            attT = streams.tile([128, 512], BF16_DT, tag="attT")
            nc.vector.tensor_mul(attT[:], osb[:], ps_rb[:])

        with nc.named_scope("oproj"):
            tags = ["A", "B", "C", "D"]
            ps_y = []
            for e in range(HID // 512):
                if e < 4:
                    ps_y.append(psA.tile([128, 512], FP32, tag=tags[e],
                                         name=f"ps_y{e}"))
                else:
                    ps_y.append(psS.tile([128, 512], FP32, tag="sc",
                                         name=f"ps_y{e}"))
            for h in range(REP):
                for e in range(HID // 512):
                    nc.tensor.matmul(
                        ps_y[e][:], attT[:, h * 128:(h + 1) * 128],
                        wo_h[h][:, e, :],
                        start=(h == 0), stop=(h == REP - 1))
            for e in range(HID // 512):
                ysb = ysbp.tile([128, 512], FP32, tag="ysb", name=f"ysb{e}")
                nc.vector.tensor_copy(ysb[:], ps_y[e][:])
                nc.scalar.dma_start(y[:, e * 512:(e + 1) * 512], ysb[:])


def _prepare_inputs(x, x_ctx, cos_q, sin_q, cos_k, sin_k, kv_cache,
                    causal_mask, Wq, Wk, Wv, Wo, q_norm_w, k_norm_w):
    """Host-side sharding/preprocessing. Returns list of per-core in_maps."""
    f32 = np.float32
    x = np.asarray(x, f32)
    x_ctx = np.asarray(x_ctx, f32)
    c = np.concatenate([x_ctx[0], x[0]], axis=0)          # [T, HID]
    cT = np.ascontiguousarray(c.T).astype(BF16)           # [HID, T]

    # x.T packed [p, (k 128l)]: xTp[p, k*128+l] = c.T[k*128+p, T-L+l]
    xTp = np.ascontiguousarray(
        c.T[:, T - L:T].reshape(KT, 128, L).transpose(1, 0, 2)
        .reshape(128, KT * L)).astype(BF16)

    # final-tile multiplicative mask: allowed iff s_local <= l,
    # replicated across the 4 q heads -> [s_local, (h l)]
    tri = (np.arange(128)[:, None] <= np.arange(128)[None, :]).astype(f32)
    triP = np.ascontiguousarray(np.tile(tri, (1, REP))).astype(BF16)

    qw = np.asarray(q_norm_w, f32).reshape(D)
    kw = np.asarray(k_norm_w, f32).reshape(D)
    rot_src = (np.arange(D) + HALF) % D                   # rotate-half source

    cosqT = np.asarray(cos_q, f32)[0, 0].T * SCALE * qw[:, None]
    sinqT = np.asarray(sin_q, f32)[0, 0].T.copy()
    sinqT[:HALF] = -sinqT[:HALF]
    sinqT = sinqT * SCALE * qw[rot_src][:, None]
    coskT = np.asarray(cos_k, f32)[0, 0].T * kw[:, None]
    sinkT = np.asarray(sin_k, f32)[0, 0].T.copy()
    sinkT[:HALF] = -sinkT[:HALF]
    sinkT = sinkT * kw[rot_src][:, None]

    Wq = np.asarray(Wq, f32)
    Wk = np.asarray(Wk, f32)
    Wv = np.asarray(Wv, f32)
    Wo = np.asarray(Wo, f32)
    kv = np.asarray(kv_cache, f32)

    in_maps = []
    for cidx in range(NCORES):
        hd = slice(cidx * REP * D, (cidx + 1) * REP * D)
        wq_c = Wq[hd].reshape(REP, D, HID)
        wq_c = wq_c - wq_c.mean(axis=1, keepdims=True)    # fold mean-subtract
        wq_c = wq_c.reshape(REP * D, HID)
        wk_c = Wk[cidx * D:(cidx + 1) * D]
        wk_c = wk_c - wk_c.mean(axis=0, keepdims=True)
        wv_c = Wv[cidx * D:(cidx + 1) * D]
        wkvT = np.concatenate([wk_c.T, wv_c.T], axis=1)   # [HID, 256]
        wqTc = np.ascontiguousarray(wq_c.T)               # [HID, 512]
        # wo packed [p, h, e_chunk, 512]: woP[p,h,j,e'] = Wo.T[h*128+p, j*512+e']
        woTc = Wo[:, hd].T.reshape(REP, 128, HID // 512, 512)
        woP = np.ascontiguousarray(woTc.transpose(1, 0, 2, 3))
        ktold = np.ascontiguousarray(kv[0, cidx, T:, :].T)  # [D, SOLD]
        # vold packed [s_local, (tile d)]: voldP[p, n*128+d] = v[n*128+p, d]
        voldP = np.ascontiguousarray(
            kv[1, cidx, T:, :].reshape(SOLD // 128, 128, D)
            .transpose(1, 0, 2).reshape(128, SOLD))
        in_maps.append(dict(
            cT=cT,
            wkvT=np.ascontiguousarray(wkvT).astype(BF16),
            wqT=wqTc.astype(BF16),
            xTp=xTp,
            woP=woP.astype(BF16),
            ktold=ktold.astype(BF16),
            voldP=voldP.astype(BF16),
            identb2=np.eye(128, dtype=f32).astype(BF16),
            triP=triP,
            cosq=cosqT.astype(BF16), sinq=sinqT.astype(BF16),
            cosk=coskT.astype(BF16), sink=sinkT.astype(BF16),
        ))
    return in_maps


def kernel(**inputs) -> np.ndarray:
    global LAST_RESULTS
    if "nc" not in _PROGRAM_CACHE:
        _PROGRAM_CACHE["nc"] = _build_program()
    nc = _PROGRAM_CACHE["nc"]
    in_maps = _prepare_inputs(**inputs)
    trace = bool(int(os.environ.get("BASS_KERNEL_TRACE", "0")))
    res = run_bass_kernel_spmd(nc, in_maps, list(range(NCORES)), trace=trace)
    LAST_RESULTS = res
    y = np.zeros((L, HID), np.float64)
    for cidx in range(NCORES):
        y += res.results[cidx]["y"].astype(np.float64)
    return y.astype(np.float32).reshape(1, L, HID)


# revision 13
# speedup vs baseline: 1.3707x; 1.0773x over previous
"""Trainium2 Bass kernel for nn_DFlashAttentionSlide (GQA attention block).

Sharding: tensor-parallel over heads across 8 NeuronCores. Core c owns
kv head c and q heads [4c, 4c+4). Activations (x/x_ctx) are replicated;
weights / kv-cache are sharded along the head dim; the output projection
is contraction-sharded, so each core returns a partial [L, HID] output
that the host sums.

Device-side schedule (per core). ~270 bf16 matmuls (mostly N=512) are
the PE floor and ~22 MB of HBM traffic is the DMA floor; the schedule
overlaps both:

  stage 1  q projection (resident wq, one 4.2MB DMA), q rmsnorm (DVE)
           -> bf16 transposes (PE) -> rope (DVE).
  stage 2  s-loop over the 24 cached stream tiles: scoresT[s,(h l)] =
           K-tile^ @ qT; exp on ACT straight PSUM->SBUF bf16 (3
           rotating banks); softmax denominator accumulated on DVE
           (fp32 += bf16 exp tile); causal mask is a triangle multiply
           on the final tile only.  PV accumulates in a dedicated bank.
  stage 3  cT streams in 8 one-MB chunks; each chunk immediately feeds
           k0/k1/v0/v1 matmuls (4 PSUM accumulators), interleaving with
           the ACT-paced s-loop.  k-norm + rope after the last chunk
           (norm weights and SCALE folded into host rope tables,
           std broadcast by bf16 ones-matmul, fast approx reciprocal);
           v transposes on PE overlap the k-norm DVE chain.
  stage 4  s-loop over the 8 new tiles, then normalize + output
           projection head-outer across all 8 PSUM banks; y chunks
           stream out on the second ring.

All matmuls bf16 (fp32 runs at 1/4 rate).  ACT runs only exp + two
sqrts so its Exp table stays loaded through each s-loop; casts are DVE.
Every DMA is a contiguous >=2KB-per-partition 2D transfer (host packs
the layouts), big (1-4MB), and ring-ordered by need-time.
"""

import os
import sys

sys.path.insert(0, "/opt/trn_rl_repo")

import numpy as np
import ml_dtypes

import concourse.bass as bass
import concourse.bacc as bacc
import concourse.tile as tile
from concourse import mybir
from concourse.bass_utils import run_bass_kernel_spmd

BF16 = ml_dtypes.bfloat16

H, HKV, D, HALF = 32, 8, 128, 64
L, T, S, HID = 128, 1024, 4096, 4096
REP = H // HKV          # q heads per kv head (= per core)
EPS = 1e-6
SCALE = D ** -0.5
NCORES = 8
KT = HID // 128         # 32 contraction tiles for projections
ST = S // 128           # 32 s tiles for attention
SOLD = S - T            # 3072 cached stream positions kept
TNEW = T                # 1024 newly projected stream positions
STOLD = SOLD // 128     # 24 cached s tiles
STNEW = TNEW // 128     # 8 new s tiles

FP32 = mybir.dt.float32
BF16_DT = mybir.dt.bfloat16

_PROGRAM_CACHE = {}

# Filled by kernel() when BASS_KERNEL_TRACE=1; read by test.py.
LAST_RESULTS = None


def _build_program():
    nc = bacc.Bacc("TRN2", target_bir_lowering=False, debug=False,
                   num_devices=NCORES)

    # ---- external I/O (per-core values supplied via in_maps) ----
    # all big tensors host-packed so every DMA row is contiguous >=2KB
    cTP = nc.declare_dram_parameter("cTP", [128, KT, T], BF16_DT, isOutput=False)
    wkvP = nc.declare_dram_parameter("wkvP", [128, KT, 256], BF16_DT, isOutput=False)
    wqP = nc.declare_dram_parameter("wqP", [128, KT, 512], BF16_DT, isOutput=False)
    xTp = nc.declare_dram_parameter("xTp", [128, KT * 128], BF16_DT, isOutput=False)
    woP = nc.declare_dram_parameter("woP", [128, REP, HID // 512, 512], BF16_DT, isOutput=False)
    ktold = nc.declare_dram_parameter("ktold", [D, SOLD], BF16_DT, isOutput=False)
    voldP = nc.declare_dram_parameter("voldP", [128, SOLD], BF16_DT, isOutput=False)
    identb2 = nc.declare_dram_parameter("identb2", [128, 128], BF16_DT, isOutput=False)
    triP = nc.declare_dram_parameter("triP", [128, 512], BF16_DT, isOutput=False)
    cosq = nc.declare_dram_parameter("cosq", [D, L], BF16_DT, isOutput=False)
    sinq = nc.declare_dram_parameter("sinq", [D, L], BF16_DT, isOutput=False)
    cosk = nc.declare_dram_parameter("cosk", [D, TNEW], BF16_DT, isOutput=False)
    sink = nc.declare_dram_parameter("sink", [D, TNEW], BF16_DT, isOutput=False)
    y = nc.declare_dram_parameter("y", [L, HID], FP32, isOutput=True)

    with tile.TileContext(nc) as tc:
        _emit(nc, tc, cTP=cTP, wkvP=wkvP, wqP=wqP, xTp=xTp, woP=woP,
              ktold=ktold, voldP=voldP, identb2=identb2, triP=triP,
              cosq=cosq, sinq=sinq, cosk=cosk, sink=sink, y=y)
    nc.compile()
    return nc


def _emit(nc, tc, *, cTP, wkvP, wqP, xTp, woP, ktold, voldP, identb2, triP,
          cosq, sinq, cosk, sink, y):
    from contextlib import ExitStack

    ctx = ExitStack()
    with ctx:
        # ---------------- pools ----------------
        consts = ctx.enter_context(tc.tile_pool(name="consts", bufs=1))
        streams = ctx.enter_context(tc.tile_pool(name="streams", bufs=1))
        ctp = ctx.enter_context(tc.tile_pool(name="ctp", bufs=4))
        wop = ctx.enter_context(tc.tile_pool(name="wop", bufs=4))
        ntmp = ctx.enter_context(tc.tile_pool(name="ntmp", bufs=1))
        sloop = ctx.enter_context(tc.tile_pool(name="sloop", bufs=6))
        ysbp = ctx.enter_context(tc.tile_pool(name="ysbp", bufs=2))
        psA = ctx.enter_context(tc.tile_pool(name="psA", bufs=1, space="PSUM"))
        psS = ctx.enter_context(tc.tile_pool(name="psS", bufs=3, space="PSUM"))

        # ---------------- constants ----------------
        ones_colb = consts.tile([128, 1], BF16_DT, tag="ones_colb")
        nc.vector.memset(ones_colb, 1.0)
        ones_colf = consts.tile([128, 1], FP32, tag="ones_colf")
        nc.vector.memset(ones_colf, 1.0)
        ones_rowb = consts.tile([1, 128], BF16_DT, tag="ones_rowb")
        nc.vector.memset(ones_rowb, 1.0)
        eps_t = consts.tile([128, 1], FP32, tag="eps")
        nc.vector.memset(eps_t, EPS)
        den_acc = streams.tile([128, 512], FP32, tag="den_acc")
        nc.vector.memset(den_acc, 0.0)

        # small early residents on the qAct ring (ACT is idle until exp)
        identb = consts.tile([128, 128], BF16_DT, tag="identb")
        nc.scalar.dma_start(identb[:], identb2[:])
        cosq_t = consts.tile([D, L], BF16_DT, tag="cosq")
        nc.scalar.dma_start(cosq_t[:], cosq[:])
        sinq_t = consts.tile([D, L], BF16_DT, tag="sinq")
        nc.scalar.dma_start(sinq_t[:], sinq[:])
        tri_t = consts.tile([128, 512], BF16_DT, tag="tri")
        nc.scalar.dma_start(tri_t[:], triP[:])
        vt_oa = streams.tile([128, SOLD // 2], BF16_DT, tag="vt_oa")
        nc.scalar.dma_start(vt_oa[:], voldP[:, 0:SOLD // 2])
        vt_ob = streams.tile([128, SOLD // 2], BF16_DT, tag="vt_ob")
        nc.scalar.dma_start(vt_ob[:], voldP[:, SOLD // 2:SOLD])
        wkv_res = streams.tile([128, KT, 256], BF16_DT, tag="wkv")
        nc.scalar.dma_start(wkv_res[:], wkvP[:])
        cosk_t = consts.tile([D, TNEW], BF16_DT, tag="cosk")
        nc.scalar.dma_start(cosk_t[:], cosk[:])
        sink_t = consts.tile([D, TNEW], BF16_DT, tag="sink")
        nc.scalar.dma_start(sink_t[:], sink[:])

        # ---------------- stage 1: q path (qSP ring) ----------------
        xT_res = streams.tile([128, KT * 128], BF16_DT, tag="xT")
        nc.sync.dma_start(xT_res[:], xTp[:])
        wq_res = streams.tile([128, KT, 512], BF16_DT, tag="wq")
        nc.sync.dma_start(wq_res[:], wqP[:])
        # cached k stream tiles, split so attention starts after half lands
        kts_oa = streams.tile([128, SOLD // 2], BF16_DT, tag="kts_oa")
        nc.sync.dma_start(kts_oa[:], ktold[:, 0:SOLD // 2])
        kts_ob = streams.tile([128, SOLD // 2], BF16_DT, tag="kts_ob")
        nc.sync.dma_start(kts_ob[:], ktold[:, SOLD // 2:SOLD])

        ps_q = psA.tile([128, 512], FP32, tag="A")
        with nc.named_scope("qproj"):
            for k in range(KT):
                nc.tensor.matmul(ps_q[:],
                                 xT_res[:, k * 128:(k + 1) * 128],
                                 wq_res[:, k, :],
                                 start=(k == 0), stop=(k == KT - 1))

        with nc.named_scope("qnorm"):
            qsb = ntmp.tile([128, 512], FP32, tag="qsb")
            nc.vector.tensor_copy(qsb[:], ps_q[:])
            qsq = ntmp.tile([128, 512], FP32, tag="qsq")
            nc.vector.tensor_mul(qsq[:], qsb[:], qsb[:])
            qsos = ntmp.tile([128, REP], FP32, tag="qsos")
            nc.vector.reduce_sum(
                qsos[:],
                qsq[:].rearrange("p (h d) -> p h d", h=REP),
                axis=mybir.AxisListType.X,
            )
            qstd = ntmp.tile([128, REP], FP32, tag="qstd")
            nc.scalar.activation(qstd[:], qsos[:],
                                 mybir.ActivationFunctionType.Sqrt,
                                 bias=eps_t[:], scale=1.0 / D)
            qrstd = ntmp.tile([128, REP], FP32, tag="qrstd")
            nc.vector.reciprocal_approx_fast(out=qrstd[:], in_=qstd[:])
            qn = ntmp.tile([128, 512], BF16_DT, tag="qn")
            for h in range(REP):
                nc.vector.tensor_scalar_mul(qn[:, h * 128:(h + 1) * 128],
                                            qsb[:, h * 128:(h + 1) * 128],
                                            qrstd[:, h:h + 1])
            # bf16 PE transposes -> qT layout [d, (h l)]
            qtw = ntmp.tile([128, 512], BF16_DT, tag="qtw")
            for h in range(REP):
                ps_qT = psS.tile([128, 128], BF16_DT, tag="sc")
                nc.tensor.transpose(ps_qT[:], qn[:, h * 128:(h + 1) * 128],
                                    identb[:])
                nc.vector.tensor_copy(qtw[:, h * 128:(h + 1) * 128], ps_qT[:])
            # rope (sign + SCALE + q_norm_w folded into host tables)
            qrot = ntmp.tile([128, 512], BF16_DT, tag="qrot")
            nc.scalar.dma_start(qrot[0:HALF, :], qtw[HALF:D, :])
            nc.scalar.dma_start(qrot[HALF:D, :], qtw[0:HALF, :])
            qa = ntmp.tile([128, 512], BF16_DT, tag="qa")
            qb = ntmp.tile([128, 512], BF16_DT, tag="qb")
            for h in range(REP):
                sl = slice(h * 128, (h + 1) * 128)
                nc.vector.tensor_mul(qa[:, sl], qtw[:, sl], cosq_t[:])
                nc.vector.tensor_mul(qb[:, sl], qrot[:, sl], sinq_t[:])
            qT_all = streams.tile([128, 512], BF16_DT, tag="qT_all")
            nc.vector.tensor_add(qT_all[:], qa[:], qb[:])

        # ---------------- stage 2: attention s-loop ----------------
        kts_new = streams.tile([128, TNEW], BF16_DT, tag="kts_new")
        vt_new = streams.tile([128, TNEW], BF16_DT, tag="vt_new")
        ps_o = psA.tile([128, 512], FP32, tag="A")

        def s_iter(s, ksrc, vsrc):
            ps_sc = psS.tile([128, 512], FP32, tag="sc")
            nc.tensor.matmul(ps_sc[:], ksrc, qT_all[:])
            ex = sloop.tile([128, 512], BF16_DT, tag="ex")
            nc.scalar.activation(ex[:], ps_sc[:],
                                 mybir.ActivationFunctionType.Exp)
            if s == ST - 1:
                exm = ntmp.tile([128, 512], BF16_DT, tag="exm")
                nc.vector.tensor_mul(exm[:], ex[:], tri_t[:])
                ex = exm
            # softmax denominator on DVE: fp32 accumulator += bf16 exp
            nc.vector.tensor_add(den_acc[:], den_acc[:], ex[:])
            nc.tensor.matmul(ps_o[:], vsrc, ex[:],
                             start=(s == 0), stop=(s == ST - 1))

        HT = STOLD // 2
        with nc.named_scope("sloop_old"):
            for s in range(STOLD):
                if s < HT:
                    s_iter(s, kts_oa[:, s * 128:(s + 1) * 128],
                           vt_oa[:, s * 128:(s + 1) * 128])
                else:
                    j = s - HT
                    s_iter(s, kts_ob[:, j * 128:(j + 1) * 128],
                           vt_ob[:, j * 128:(j + 1) * 128])

        # ---------------- stage 3: k/v projections (streamed cT) --------
        ps_k0 = psA.tile([128, 512], FP32, tag="C")
        ps_k1 = psA.tile([128, 512], FP32, tag="D")
        ps_v0 = psA.tile([128, 512], FP32, tag="E")
        ps_v1 = psA.tile([128, 512], FP32, tag="F")
        with nc.named_scope("kvproj"):
            for g in range(8):
                ct_g = ctp.tile([128, 4, T], BF16_DT, tag="ct")
                nc.sync.dma_start(ct_g[:], cTP[:, g * 4:(g + 1) * 4, :])
                for j in range(4):
                    k = g * 4 + j
                    wk = wkv_res[:, k, 0:128]
                    wv = wkv_res[:, k, 128:256]
                    ct_sl = ct_g[:, j, :]
                    st = (k == 0)
                    sp = (k == KT - 1)
                    nc.tensor.matmul(ps_k0[:], wk, ct_sl[:, 0:512],
                                     start=st, stop=sp)
                    nc.tensor.matmul(ps_k1[:], wk, ct_sl[:, 512:1024],
                                     start=st, stop=sp)
                    nc.tensor.matmul(ps_v0[:], wv, ct_sl[:, 0:512],
                                     start=st, stop=sp)
                    nc.tensor.matmul(ps_v1[:], wv, ct_sl[:, 512:1024],
                                     start=st, stop=sp)

        # wo on the qSP ring behind cT (FIFO -> no bandwidth steal)
        wo_h = []
        for h in range(REP):
            wt = wop.tile([128, HID // 512, 512], BF16_DT, name=f"wo{h}",
                          tag="wo")
            nc.sync.dma_start(wt[:], woP[:, h, :, :])
            wo_h.append(wt)

        with nc.named_scope("knorm"):
            kc = ntmp.tile([128, TNEW], BF16_DT, tag="qsb")
            nc.vector.tensor_copy(kc[:, 0:512], ps_k0[:])
            nc.vector.tensor_copy(kc[:, 512:1024], ps_k1[:])
            vsb = ntmp.tile([128, TNEW], BF16_DT, tag="vsb")
            nc.vector.tensor_copy(vsb[:, 0:512], ps_v0[:])
            nc.vector.tensor_copy(vsb[:, 512:1024], ps_v1[:])
            ksq = ntmp.tile([128, TNEW], BF16_DT, tag="qsq")
            nc.vector.tensor_mul(ksq[:, 0:512], kc[:, 0:512], kc[:, 0:512])
            nc.vector.tensor_mul(ksq[:, 512:1024], kc[:, 512:1024],
                                 kc[:, 512:1024])
            ps_sos0 = psS.tile([1, 512], FP32, tag="sc")
            ps_sos1 = psS.tile([1, 512], FP32, tag="sc")
            nc.tensor.matmul(ps_sos0[:], ones_colb[:], ksq[:, 0:512])
            nc.tensor.matmul(ps_sos1[:], ones_colb[:], ksq[:, 512:1024])
            kstd = ntmp.tile([1, TNEW], BF16_DT, tag="kstd")
            nc.scalar.activation(kstd[:, 0:512], ps_sos0[:],
                                 mybir.ActivationFunctionType.Sqrt,
                                 bias=eps_t[0:1, :], scale=1.0 / D)
            nc.scalar.activation(kstd[:, 512:1024], ps_sos1[:],
                                 mybir.ActivationFunctionType.Sqrt,
                                 bias=eps_t[0:1, :], scale=1.0 / D)
            # broadcast std across partitions, then fast DVE reciprocal
            ps_kb0 = psS.tile([128, 512], FP32, tag="sc")
            ps_kb1 = psS.tile([128, 512], FP32, tag="sc")
            nc.tensor.matmul(ps_kb0[:], ones_rowb[:], kstd[:, 0:512])
            nc.tensor.matmul(ps_kb1[:], ones_rowb[:], kstd[:, 512:1024])
            krr = ntmp.tile([128, TNEW], FP32, tag="krr")
            nc.vector.reciprocal_approx_fast(out=krr[:, 0:512], in_=ps_kb0[:])
            nc.vector.reciprocal_approx_fast(out=krr[:, 512:1024],
                                             in_=ps_kb1[:])
            krrb = ntmp.tile([128, TNEW], BF16_DT, tag="qtw")
            nc.vector.tensor_copy(krrb[:], krr[:])
            knw = ntmp.tile([128, TNEW], BF16_DT, tag="qn")
            nc.vector.tensor_mul(knw[:], kc[:], krrb[:])
            # rope (sign + k_norm_w folded into host tables)
            krot = ntmp.tile([128, TNEW], BF16_DT, tag="qrot")
            nc.scalar.dma_start(krot[0:HALF, :], knw[HALF:D, :])
            nc.scalar.dma_start(krot[HALF:D, :], knw[0:HALF, :])
            ka = ntmp.tile([128, TNEW], BF16_DT, tag="qa")
            nc.vector.tensor_mul(ka[:], knw[:], cosk_t[:])
            kb = ntmp.tile([128, TNEW], BF16_DT, tag="qb")
            nc.vector.tensor_mul(kb[:], krot[:], sink_t[:])
            nc.vector.tensor_add(kts_new[:], ka[:], kb[:])
            # v transposes into stream layout [s_local, d] (PE, overlaps
            # the k-norm DVE chain)
            for i in range(STNEW):
                ps_vT = psS.tile([128, 128], BF16_DT, tag="sc",
                                 name=f"ps_vT{i}")
                nc.tensor.transpose(ps_vT[:], vsb[:, i * 128:(i + 1) * 128],
                                    identb[:])
                nc.vector.tensor_copy(vt_new[:, i * 128:(i + 1) * 128],
                                      ps_vT[:])

        # s-loop over the newly projected tiles
        with nc.named_scope("sloop_new"):
            for s in range(STOLD, ST):
                j = s - STOLD
                s_iter(s, kts_new[:, j * 128:(j + 1) * 128],
                       vt_new[:, j * 128:(j + 1) * 128])

        # ---------------- stage 4: normalize + output projection --------
        with nc.named_scope("fin"):
            ps_den = psS.tile([1, 512], FP32, tag="sc")
            nc.tensor.matmul(ps_den[:], ones_colf[:], den_acc[:])
            rec = ntmp.tile([1, 512], FP32, tag="rec")
            nc.vector.reciprocal_approx_fast(out=rec[:], in_=ps_den[:])
            recb = ntmp.tile([1, 512], BF16_DT, tag="recb")
            nc.vector.tensor_copy(recb[:], rec[:])
            ps_rb = psS.tile([128, 512], FP32, tag="sc")
            nc.tensor.matmul(ps_rb[:], ones_rowb[:], recb[:])
            osb = ntmp.tile([128, 512], FP32, tag="osb")
            nc.vector.tensor_copy(osb[:], ps_o[:])
            attT = streams.tile([128, 512], BF16_DT, tag="attT")
            nc.vector.tensor_mul(attT[:], osb[:], ps_rb[:])

        with nc.named_scope("oproj"):
            tags = ["A", "C", "D", "E", "F"]
            ps_y = []
            for e in range(HID // 512):
                if e < 5:
                    ps_y.append(psA.tile([128, 512], FP32, tag=tags[e],
                                         name=f"ps_y{e}"))
                else:
                    ps_y.append(psS.tile([128, 512], FP32, tag="sc",
                                         name=f"ps_y{e}"))
            for h in range(REP):
                for e in range(HID // 512):
                    nc.tensor.matmul(
                        ps_y[e][:], attT[:, h * 128:(h + 1) * 128],
                        wo_h[h][:, e, :],
                        start=(h == 0), stop=(h == REP - 1))
            for e in range(HID // 512):
                ysb = ysbp.tile([128, 512], FP32, tag="ysb", name=f"ysb{e}")
                nc.vector.tensor_copy(ysb[:], ps_y[e][:])
                nc.scalar.dma_start(y[:, e * 512:(e + 1) * 512], ysb[:])


def _prepare_inputs(x, x_ctx, cos_q, sin_q, cos_k, sin_k, kv_cache,
                    causal_mask, Wq, Wk, Wv, Wo, q_norm_w, k_norm_w):
    """Host-side sharding/preprocessing. Returns list of per-core in_maps."""
    f32 = np.float32
    x = np.asarray(x, f32)
    x_ctx = np.asarray(x_ctx, f32)
    c = np.concatenate([x_ctx[0], x[0]], axis=0)          # [T, HID]
    # cT packed [p, k, t]: cTP[p, k, t] = c.T[k*128+p, t]
    cTP = np.ascontiguousarray(
        c.T.reshape(KT, 128, T).transpose(1, 0, 2)).astype(BF16)

    # x.T packed [p, (k 128l)]: xTp[p, k*128+l] = c.T[k*128+p, T-L+l]
    xTp = np.ascontiguousarray(
        c.T[:, T - L:T].reshape(KT, 128, L).transpose(1, 0, 2)
        .reshape(128, KT * L)).astype(BF16)

    # final-tile multiplicative mask: allowed iff s_local <= l,
    # replicated across the 4 q heads -> [s_local, (h l)]
    tri = (np.arange(128)[:, None] <= np.arange(128)[None, :]).astype(f32)
    triP = np.ascontiguousarray(np.tile(tri, (1, REP))).astype(BF16)

    qw = np.asarray(q_norm_w, f32).reshape(D)
    kw = np.asarray(k_norm_w, f32).reshape(D)
    rot_src = (np.arange(D) + HALF) % D                   # rotate-half source

    cosqT = np.asarray(cos_q, f32)[0, 0].T * SCALE * qw[:, None]
    sinqT = np.asarray(sin_q, f32)[0, 0].T.copy()
    sinqT[:HALF] = -sinqT[:HALF]
    sinqT = sinqT * SCALE * qw[rot_src][:, None]
    coskT = np.asarray(cos_k, f32)[0, 0].T * kw[:, None]
    sinkT = np.asarray(sin_k, f32)[0, 0].T.copy()
    sinkT[:HALF] = -sinkT[:HALF]
    sinkT = sinkT * kw[rot_src][:, None]

    Wq = np.asarray(Wq, f32)
    Wk = np.asarray(Wk, f32)
    Wv = np.asarray(Wv, f32)
    Wo = np.asarray(Wo, f32)
    kv = np.asarray(kv_cache, f32)

    in_maps = []
    for cidx in range(NCORES):
        hd = slice(cidx * REP * D, (cidx + 1) * REP * D)
        wq_c = Wq[hd].reshape(REP, D, HID)
        wq_c = wq_c - wq_c.mean(axis=1, keepdims=True)    # fold mean-subtract
        wq_c = wq_c.reshape(REP * D, HID)
        wk_c = Wk[cidx * D:(cidx + 1) * D]
        wk_c = wk_c - wk_c.mean(axis=0, keepdims=True)
        wv_c = Wv[cidx * D:(cidx + 1) * D]
        wkvT = np.concatenate([wk_c.T, wv_c.T], axis=1)   # [HID, 256]
        # packed [p, k, n]: wkvP[p, k, n] = wkvT[k*128+p, n]
        wkvP = np.ascontiguousarray(
            wkvT.reshape(KT, 128, 256).transpose(1, 0, 2)).astype(BF16)
        wqTc = np.ascontiguousarray(wq_c.T)               # [HID, 512]
        wqP = np.ascontiguousarray(
            wqTc.reshape(KT, 128, 512).transpose(1, 0, 2)).astype(BF16)
        # wo packed [p, h, e_chunk, 512]: woP[p,h,j,e'] = Wo.T[h*128+p, j*512+e']
        woTc = Wo[:, hd].T.reshape(REP, 128, HID // 512, 512)
        woP = np.ascontiguousarray(woTc.transpose(1, 0, 2, 3))
        ktold = np.ascontiguousarray(kv[0, cidx, T:, :].T)  # [D, SOLD]
        # vold packed [s_local, (tile d)]: voldP[p, n*128+d] = v[n*128+p, d]
        voldP = np.ascontiguousarray(
            kv[1, cidx, T:, :].reshape(SOLD // 128, 128, D)
            .transpose(1, 0, 2).reshape(128, SOLD))
        in_maps.append(dict(
            cTP=cTP,
            wkvP=wkvP,
            wqP=wqP.astype(BF16),
            xTp=xTp,
            woP=woP.astype(BF16),
            ktold=ktold.astype(BF16),
            voldP=voldP.astype(BF16),
            identb2=np.eye(128, dtype=f32).astype(BF16),
            triP=triP,
            cosq=cosqT.astype(BF16), sinq=sinqT.astype(BF16),
            cosk=coskT.astype(BF16), sink=sinkT.astype(BF16),
        ))
    return in_maps


def kernel(**inputs) -> np.ndarray:
    global LAST_RESULTS
    if "nc" not in _PROGRAM_CACHE:
        _PROGRAM_CACHE["nc"] = _build_program()
    nc = _PROGRAM_CACHE["nc"]
    in_maps = _prepare_inputs(**inputs)
    trace = bool(int(os.environ.get("BASS_KERNEL_TRACE", "0")))
    res = run_bass_kernel_spmd(nc, in_maps, list(range(NCORES)), trace=trace)
    LAST_RESULTS = res
    y = np.zeros((L, HID), np.float64)
    for cidx in range(NCORES):
        y += res.results[cidx]["y"].astype(np.float64)
    return y.astype(np.float32).reshape(1, L, HID)


# revision 14
# speedup vs baseline: 1.4134x; 1.0311x over previous
"""Trainium2 Bass kernel for nn_DFlashAttentionSlide (GQA attention block).

Sharding: tensor-parallel over heads across 8 NeuronCores. Core c owns
kv head c and q heads [4c, 4c+4). Activations (x/x_ctx) are replicated;
weights / kv-cache are sharded along the head dim; the output projection
is contraction-sharded, so each core returns a partial [L, HID] output
that the host sums.

Device-side schedule (per core). ~270 bf16 matmuls (mostly N=512) are
the PE floor and ~22 MB of HBM traffic is the DMA floor; the schedule
overlaps both:

  stage 1  q projection (resident wq, one 4.2MB DMA), q rmsnorm (DVE)
           -> bf16 transposes (PE) -> rope (DVE).
  stage 2  s-loop over the 24 cached stream tiles: scoresT[s,(h l)] =
           K-tile^ @ qT; exp on ACT straight PSUM->SBUF bf16 (3
           rotating banks); softmax denominator accumulated on DVE
           (fp32 += bf16 exp tile); causal mask is a triangle multiply
           on the final tile only.  PV accumulates in a dedicated bank.
  stage 3  cT streams in 8 one-MB chunks; each chunk immediately feeds
           k0/k1/v0/v1 matmuls (4 PSUM accumulators), interleaving with
           the ACT-paced s-loop.  k-norm + rope after the last chunk
           (norm weights and SCALE folded into host rope tables,
           std broadcast by bf16 ones-matmul, fast approx reciprocal);
           v transposes on PE overlap the k-norm DVE chain.
  stage 4  s-loop over the 8 new tiles, then normalize + output
           projection head-outer across all 8 PSUM banks; y chunks
           stream out on the second ring.

All matmuls bf16 (fp32 runs at 1/4 rate).  ACT runs only exp + two
sqrts so its Exp table stays loaded through each s-loop; casts are DVE.
Every DMA is a contiguous >=2KB-per-partition 2D transfer (host packs
the layouts), big (1-4MB), and ring-ordered by need-time.
"""

import os
import sys

sys.path.insert(0, "/opt/trn_rl_repo")

import numpy as np
import ml_dtypes

import concourse.bass as bass
import concourse.bacc as bacc
import concourse.tile as tile
from concourse import mybir
from concourse.bass_utils import run_bass_kernel_spmd

BF16 = ml_dtypes.bfloat16

H, HKV, D, HALF = 32, 8, 128, 64
L, T, S, HID = 128, 1024, 4096, 4096
REP = H // HKV          # q heads per kv head (= per core)
EPS = 1e-6
SCALE = D ** -0.5
NCORES = 8
KT = HID // 128         # 32 contraction tiles for projections
ST = S // 128           # 32 s tiles for attention
SOLD = S - T            # 3072 cached stream positions kept
TNEW = T                # 1024 newly projected stream positions
STOLD = SOLD // 128     # 24 cached s tiles
STNEW = TNEW // 128     # 8 new s tiles

FP32 = mybir.dt.float32
BF16_DT = mybir.dt.bfloat16

_PROGRAM_CACHE = {}

# Filled by kernel() when BASS_KERNEL_TRACE=1; read by test.py.
LAST_RESULTS = None


def _build_program():
    nc = bacc.Bacc("TRN2", target_bir_lowering=False, debug=False,
                   num_devices=NCORES)

    # ---- external I/O (per-core values supplied via in_maps) ----
    # all big tensors host-packed so every DMA row is contiguous >=2KB
    cTP = nc.declare_dram_parameter("cTP", [128, KT, T], BF16_DT, isOutput=False)
    wkvP = nc.declare_dram_parameter("wkvP", [128, KT, 256], BF16_DT, isOutput=False)
    wqP = nc.declare_dram_parameter("wqP", [128, KT, 512], BF16_DT, isOutput=False)
    xTp = nc.declare_dram_parameter("xTp", [128, KT * 128], BF16_DT, isOutput=False)
    woP = nc.declare_dram_parameter("woP", [128, REP, HID // 512, 512], BF16_DT, isOutput=False)
    ktold = nc.declare_dram_parameter("ktold", [D, SOLD], BF16_DT, isOutput=False)
    voldP = nc.declare_dram_parameter("voldP", [128, SOLD], BF16_DT, isOutput=False)
    identb2 = nc.declare_dram_parameter("identb2", [128, 128], BF16_DT, isOutput=False)
    rotP = nc.declare_dram_parameter("rotP", [128, 128], BF16_DT, isOutput=False)
    triP = nc.declare_dram_parameter("triP", [128, 512], BF16_DT, isOutput=False)
    cosq = nc.declare_dram_parameter("cosq", [D, L], BF16_DT, isOutput=False)
    sinq = nc.declare_dram_parameter("sinq", [D, L], BF16_DT, isOutput=False)
    cosk = nc.declare_dram_parameter("cosk", [D, TNEW], BF16_DT, isOutput=False)
    sink = nc.declare_dram_parameter("sink", [D, TNEW], BF16_DT, isOutput=False)
    y = nc.declare_dram_parameter("y", [L, HID], FP32, isOutput=True)

    with tile.TileContext(nc) as tc:
        _emit(nc, tc, cTP=cTP, wkvP=wkvP, wqP=wqP, xTp=xTp, woP=woP,
              ktold=ktold, voldP=voldP, identb2=identb2, rotP=rotP,
              triP=triP, cosq=cosq, sinq=sinq, cosk=cosk, sink=sink, y=y)
    nc.compile()
    return nc


def _emit(nc, tc, *, cTP, wkvP, wqP, xTp, woP, ktold, voldP, identb2,
          rotP, triP, cosq, sinq, cosk, sink, y):
    from contextlib import ExitStack

    ctx = ExitStack()
    with ctx:
        # ---------------- pools ----------------
        consts = ctx.enter_context(tc.tile_pool(name="consts", bufs=1))
        streams = ctx.enter_context(tc.tile_pool(name="streams", bufs=1))
        ctp = ctx.enter_context(tc.tile_pool(name="ctp", bufs=4))
        wop = ctx.enter_context(tc.tile_pool(name="wop", bufs=4))
        ntmp = ctx.enter_context(tc.tile_pool(name="ntmp", bufs=1))
        sloop = ctx.enter_context(tc.tile_pool(name="sloop", bufs=6))
        ysbp = ctx.enter_context(tc.tile_pool(name="ysbp", bufs=2))
        psA = ctx.enter_context(tc.tile_pool(name="psA", bufs=1, space="PSUM"))
        psS = ctx.enter_context(tc.tile_pool(name="psS", bufs=3, space="PSUM"))

        # ---------------- constants ----------------
        ones_colb = consts.tile([128, 1], BF16_DT, tag="ones_colb")
        nc.vector.memset(ones_colb, 1.0)
        ones_colf = consts.tile([128, 1], FP32, tag="ones_colf")
        nc.vector.memset(ones_colf, 1.0)
        ones_rowb = consts.tile([1, 128], BF16_DT, tag="ones_rowb")
        nc.vector.memset(ones_rowb, 1.0)
        eps_t = consts.tile([128, 1], FP32, tag="eps")
        nc.vector.memset(eps_t, EPS)
        den_acc = streams.tile([128, 512], FP32, tag="den_acc")
        nc.vector.memset(den_acc, 0.0)

        # small early residents on the qAct ring (ACT is idle until exp)
        identb = consts.tile([128, 128], BF16_DT, tag="identb")
        nc.scalar.dma_start(identb[:], identb2[:])
        rot_t = consts.tile([128, 128], BF16_DT, tag="rot")
        nc.scalar.dma_start(rot_t[:], rotP[:])
        cosq_t = consts.tile([D, L], BF16_DT, tag="cosq")
        nc.scalar.dma_start(cosq_t[:], cosq[:])
        sinq_t = consts.tile([D, L], BF16_DT, tag="sinq")
        nc.scalar.dma_start(sinq_t[:], sinq[:])
        tri_t = consts.tile([128, 512], BF16_DT, tag="tri")
        nc.scalar.dma_start(tri_t[:], triP[:])
        # ---------------- stage 1: q path (both rings) --------------
        xT_res = streams.tile([128, KT * 128], BF16_DT, tag="xT")
        nc.sync.dma_start(xT_res[:], xTp[:])
        wq_res = streams.tile([128, KT, 512], BF16_DT, tag="wq")
        nc.sync.dma_start(wq_res[:, 0:KT // 2, :], wqP[:, 0:KT // 2, :])
        nc.scalar.dma_start(wq_res[:, KT // 2:KT, :], wqP[:, KT // 2:KT, :])
        # cached k/v stream tiles, split so attention starts early
        kts_oa = streams.tile([128, SOLD // 2], BF16_DT, tag="kts_oa")
        nc.sync.dma_start(kts_oa[:], ktold[:, 0:SOLD // 2])
        kts_ob = streams.tile([128, SOLD // 2], BF16_DT, tag="kts_ob")
        nc.sync.dma_start(kts_ob[:], ktold[:, SOLD // 2:SOLD])
        vt_oa = streams.tile([128, SOLD // 2], BF16_DT, tag="vt_oa")
        nc.scalar.dma_start(vt_oa[:], voldP[:, 0:SOLD // 2])
        vt_ob = streams.tile([128, SOLD // 2], BF16_DT, tag="vt_ob")
        nc.scalar.dma_start(vt_ob[:], voldP[:, SOLD // 2:SOLD])
        wkv_res = streams.tile([128, KT, 256], BF16_DT, tag="wkv")
        nc.scalar.dma_start(wkv_res[:], wkvP[:])
        cosk_t = consts.tile([D, TNEW], BF16_DT, tag="cosk")
        nc.scalar.dma_start(cosk_t[:], cosk[:])
        sink_t = consts.tile([D, TNEW], BF16_DT, tag="sink")
        nc.scalar.dma_start(sink_t[:], sink[:])

        ps_q = psA.tile([128, 512], FP32, tag="A")
        with nc.named_scope("qproj"):
            for k in range(KT):
                nc.tensor.matmul(ps_q[:],
                                 xT_res[:, k * 128:(k + 1) * 128],
                                 wq_res[:, k, :],
                                 start=(k == 0), stop=(k == KT - 1))

        with nc.named_scope("qnorm"):
            qsb = ntmp.tile([128, 512], FP32, tag="qsb")
            nc.vector.tensor_copy(qsb[:], ps_q[:])
            qsq = ntmp.tile([128, 512], FP32, tag="qsq")
            nc.vector.tensor_mul(qsq[:], qsb[:], qsb[:])
            qsos = ntmp.tile([128, REP], FP32, tag="qsos")
            nc.vector.reduce_sum(
                qsos[:],
                qsq[:].rearrange("p (h d) -> p h d", h=REP),
                axis=mybir.AxisListType.X,
            )
            qstd = ntmp.tile([128, REP], FP32, tag="qstd")
            nc.scalar.activation(qstd[:], qsos[:],
                                 mybir.ActivationFunctionType.Sqrt,
                                 bias=eps_t[:], scale=1.0 / D)
            qrstd = ntmp.tile([128, REP], FP32, tag="qrstd")
            nc.vector.reciprocal_approx_fast(out=qrstd[:], in_=qstd[:])
            qn = ntmp.tile([128, 512], BF16_DT, tag="qn")
            for h in range(REP):
                nc.vector.tensor_scalar_mul(qn[:, h * 128:(h + 1) * 128],
                                            qsb[:, h * 128:(h + 1) * 128],
                                            qrstd[:, h:h + 1])
            # bf16 PE transposes -> qT layout [d, (h l)]
            qtw = ntmp.tile([128, 512], BF16_DT, tag="qtw")
            for h in range(REP):
                ps_qT = psS.tile([128, 128], BF16_DT, tag="sc")
                nc.tensor.transpose(ps_qT[:], qn[:, h * 128:(h + 1) * 128],
                                    identb[:])
                nc.vector.tensor_copy(qtw[:, h * 128:(h + 1) * 128], ps_qT[:])
            # rope (sign + SCALE + q_norm_w folded into host tables);
            # rotate-half = permutation matmul (no ring dependency)
            ps_qr = psS.tile([128, 512], FP32, tag="sc")
            nc.tensor.matmul(ps_qr[:], rot_t[:], qtw[:])
            qrot = ntmp.tile([128, 512], BF16_DT, tag="qrot")
            nc.vector.tensor_copy(qrot[:], ps_qr[:])
            qa = ntmp.tile([128, 512], BF16_DT, tag="qa")
            qb = ntmp.tile([128, 512], BF16_DT, tag="qb")
            for h in range(REP):
                sl = slice(h * 128, (h + 1) * 128)
                nc.vector.tensor_mul(qa[:, sl], qtw[:, sl], cosq_t[:])
                nc.vector.tensor_mul(qb[:, sl], qrot[:, sl], sinq_t[:])
            qT_all = streams.tile([128, 512], BF16_DT, tag="qT_all")
            nc.vector.tensor_add(qT_all[:], qa[:], qb[:])

        # ---------------- stage 2: attention s-loop ----------------
        kts_new = streams.tile([128, TNEW], BF16_DT, tag="kts_new")
        vt_new = streams.tile([128, TNEW], BF16_DT, tag="vt_new")
        ps_o = psA.tile([128, 512], FP32, tag="A")

        def s_iter(s, ksrc, vsrc):
            ps_sc = psS.tile([128, 512], FP32, tag="sc")
            nc.tensor.matmul(ps_sc[:], ksrc, qT_all[:])
            ex = sloop.tile([128, 512], BF16_DT, tag="ex")
            nc.scalar.activation(ex[:], ps_sc[:],
                                 mybir.ActivationFunctionType.Exp)
            if s == ST - 1:
                exm = ntmp.tile([128, 512], BF16_DT, tag="exm")
                nc.vector.tensor_mul(exm[:], ex[:], tri_t[:])
                ex = exm
            # softmax denominator on DVE: fp32 accumulator += bf16 exp
            nc.vector.tensor_add(den_acc[:], den_acc[:], ex[:])
            nc.tensor.matmul(ps_o[:], vsrc, ex[:],
                             start=(s == 0), stop=(s == ST - 1))

        HT = STOLD // 2
        with nc.named_scope("sloop_old"):
            for s in range(STOLD):
                if s < HT:
                    s_iter(s, kts_oa[:, s * 128:(s + 1) * 128],
                           vt_oa[:, s * 128:(s + 1) * 128])
                else:
                    j = s - HT
                    s_iter(s, kts_ob[:, j * 128:(j + 1) * 128],
                           vt_ob[:, j * 128:(j + 1) * 128])

        # ---------------- stage 3: k/v projections (streamed cT) --------
        ps_k0 = psA.tile([128, 512], FP32, tag="C")
        ps_k1 = psA.tile([128, 512], FP32, tag="D")
        ps_v0 = psA.tile([128, 512], FP32, tag="E")
        ps_v1 = psA.tile([128, 512], FP32, tag="F")
        with nc.named_scope("kvproj"):
            for g in range(8):
                ct_g = ctp.tile([128, 4, T], BF16_DT, tag="ct")
                nc.sync.dma_start(ct_g[:], cTP[:, g * 4:(g + 1) * 4, :])
                for j in range(4):
                    k = g * 4 + j
                    wk = wkv_res[:, k, 0:128]
                    wv = wkv_res[:, k, 128:256]
                    ct_sl = ct_g[:, j, :]
                    st = (k == 0)
                    sp = (k == KT - 1)
                    nc.tensor.matmul(ps_k0[:], wk, ct_sl[:, 0:512],
                                     start=st, stop=sp)
                    nc.tensor.matmul(ps_k1[:], wk, ct_sl[:, 512:1024],
                                     start=st, stop=sp)
                    nc.tensor.matmul(ps_v0[:], wv, ct_sl[:, 0:512],
                                     start=st, stop=sp)
                    nc.tensor.matmul(ps_v1[:], wv, ct_sl[:, 512:1024],
                                     start=st, stop=sp)

        # wo on the qAct ring (behind the early residents, before y)
        wo_h = []
        for h in range(REP):
            wt = wop.tile([128, HID // 512, 512], BF16_DT, name=f"wo{h}",
                          tag="wo")
            nc.scalar.dma_start(wt[:], woP[:, h, :, :])
            wo_h.append(wt)

        with nc.named_scope("knorm"):
            kc = ntmp.tile([128, TNEW], BF16_DT, tag="qsb")
            nc.vector.tensor_copy(kc[:, 0:512], ps_k0[:])
            nc.vector.tensor_copy(kc[:, 512:1024], ps_k1[:])
            vsb = ntmp.tile([128, TNEW], BF16_DT, tag="vsb")
            nc.vector.tensor_copy(vsb[:, 0:512], ps_v0[:])
            nc.vector.tensor_copy(vsb[:, 512:1024], ps_v1[:])
            ksq = ntmp.tile([128, TNEW], BF16_DT, tag="qsq")
            nc.vector.tensor_mul(ksq[:, 0:512], kc[:, 0:512], kc[:, 0:512])
            nc.vector.tensor_mul(ksq[:, 512:1024], kc[:, 512:1024],
                                 kc[:, 512:1024])
            ps_sos0 = psS.tile([1, 512], FP32, tag="sc")
            ps_sos1 = psS.tile([1, 512], FP32, tag="sc")
            nc.tensor.matmul(ps_sos0[:], ones_colb[:], ksq[:, 0:512])
            nc.tensor.matmul(ps_sos1[:], ones_colb[:], ksq[:, 512:1024])
            kstd = ntmp.tile([1, TNEW], BF16_DT, tag="kstd")
            nc.scalar.activation(kstd[:, 0:512], ps_sos0[:],
                                 mybir.ActivationFunctionType.Sqrt,
                                 bias=eps_t[0:1, :], scale=1.0 / D)
            nc.scalar.activation(kstd[:, 512:1024], ps_sos1[:],
                                 mybir.ActivationFunctionType.Sqrt,
                                 bias=eps_t[0:1, :], scale=1.0 / D)
            # broadcast std across partitions, then fast DVE reciprocal
            ps_kb0 = psS.tile([128, 512], FP32, tag="sc")
            ps_kb1 = psS.tile([128, 512], FP32, tag="sc")
            nc.tensor.matmul(ps_kb0[:], ones_rowb[:], kstd[:, 0:512])
            nc.tensor.matmul(ps_kb1[:], ones_rowb[:], kstd[:, 512:1024])
            krr = ntmp.tile([128, TNEW], FP32, tag="krr")
            nc.vector.reciprocal_approx_fast(out=krr[:, 0:512], in_=ps_kb0[:])
            nc.vector.reciprocal_approx_fast(out=krr[:, 512:1024],
                                             in_=ps_kb1[:])
            krrb = ntmp.tile([128, TNEW], BF16_DT, tag="qtw")
            nc.vector.tensor_copy(krrb[:], krr[:])
            knw = ntmp.tile([128, TNEW], BF16_DT, tag="qn")
            nc.vector.tensor_mul(knw[:], kc[:], krrb[:])
            # rope (sign + k_norm_w folded into host tables);
            # rotate-half = permutation matmuls
            ps_kr0 = psS.tile([128, 512], FP32, tag="sc")
            ps_kr1 = psS.tile([128, 512], FP32, tag="sc")
            nc.tensor.matmul(ps_kr0[:], rot_t[:], knw[:, 0:512])
            nc.tensor.matmul(ps_kr1[:], rot_t[:], knw[:, 512:1024])
            krot = ntmp.tile([128, TNEW], BF16_DT, tag="qrot")
            nc.vector.tensor_copy(krot[:, 0:512], ps_kr0[:])
            nc.vector.tensor_copy(krot[:, 512:1024], ps_kr1[:])
            ka = ntmp.tile([128, TNEW], BF16_DT, tag="qa")
            nc.vector.tensor_mul(ka[:], knw[:], cosk_t[:])
            kb = ntmp.tile([128, TNEW], BF16_DT, tag="qb")
            nc.vector.tensor_mul(kb[:], krot[:], sink_t[:])
            nc.vector.tensor_add(kts_new[:], ka[:], kb[:])
            # v transposes into stream layout [s_local, d] (PE, overlaps
            # the k-norm DVE chain)
            for i in range(STNEW):
                ps_vT = psS.tile([128, 128], BF16_DT, tag="sc",
                                 name=f"ps_vT{i}")
                nc.tensor.transpose(ps_vT[:], vsb[:, i * 128:(i + 1) * 128],
                                    identb[:])
                nc.vector.tensor_copy(vt_new[:, i * 128:(i + 1) * 128],
                                      ps_vT[:])

        # s-loop over the newly projected tiles
        with nc.named_scope("sloop_new"):
            for s in range(STOLD, ST):
                j = s - STOLD
                s_iter(s, kts_new[:, j * 128:(j + 1) * 128],
                       vt_new[:, j * 128:(j + 1) * 128])

        # ---------------- stage 4: normalize + output projection --------
        with nc.named_scope("fin"):
            ps_den = psS.tile([1, 512], FP32, tag="sc")
            nc.tensor.matmul(ps_den[:], ones_colf[:], den_acc[:])
            rec = ntmp.tile([1, 512], FP32, tag="rec")
            nc.vector.reciprocal_approx_fast(out=rec[:], in_=ps_den[:])
            recb = ntmp.tile([1, 512], BF16_DT, tag="recb")
            nc.vector.tensor_copy(recb[:], rec[:])
            ps_rb = psS.tile([128, 512], FP32, tag="sc")
            nc.tensor.matmul(ps_rb[:], ones_rowb[:], recb[:])
            osb = ntmp.tile([128, 512], FP32, tag="osb")
            nc.vector.tensor_copy(osb[:], ps_o[:])
            attT = streams.tile([128, 512], BF16_DT, tag="attT")
            nc.vector.tensor_mul(attT[:], osb[:], ps_rb[:])

        with nc.named_scope("oproj"):
            tags = ["A", "C", "D", "E", "F"]
            ps_y = []
            for e in range(HID // 512):
                if e < 5:
                    ps_y.append(psA.tile([128, 512], FP32, tag=tags[e],
                                         name=f"ps_y{e}"))
                else:
                    ps_y.append(psS.tile([128, 512], FP32, tag="sc",
                                         name=f"ps_y{e}"))
            for h in range(REP):
                for e in range(HID // 512):
                    nc.tensor.matmul(
                        ps_y[e][:], attT[:, h * 128:(h + 1) * 128],
                        wo_h[h][:, e, :],
                        start=(h == 0), stop=(h == REP - 1))
            for e in range(HID // 512):
                ysb = ysbp.tile([128, 512], FP32, tag="ysb", name=f"ysb{e}")
                nc.vector.tensor_copy(ysb[:], ps_y[e][:])
                nc.scalar.dma_start(y[:, e * 512:(e + 1) * 512], ysb[:])


def _prepare_inputs(x, x_ctx, cos_q, sin_q, cos_k, sin_k, kv_cache,
                    causal_mask, Wq, Wk, Wv, Wo, q_norm_w, k_norm_w):
    """Host-side sharding/preprocessing. Returns list of per-core in_maps."""
    f32 = np.float32
    x = np.asarray(x, f32)
    x_ctx = np.asarray(x_ctx, f32)
    c = np.concatenate([x_ctx[0], x[0]], axis=0)          # [T, HID]
    # cT packed [p, k, t]: cTP[p, k, t] = c.T[k*128+p, t]
    cTP = np.ascontiguousarray(
        c.T.reshape(KT, 128, T).transpose(1, 0, 2)).astype(BF16)

    # x.T packed [p, (k 128l)]: xTp[p, k*128+l] = c.T[k*128+p, T-L+l]
    xTp = np.ascontiguousarray(
        c.T[:, T - L:T].reshape(KT, 128, L).transpose(1, 0, 2)
        .reshape(128, KT * L)).astype(BF16)

    # final-tile multiplicative mask: allowed iff s_local <= l,
    # replicated across the 4 q heads -> [s_local, (h l)]
    tri = (np.arange(128)[:, None] <= np.arange(128)[None, :]).astype(f32)
    triP = np.ascontiguousarray(np.tile(tri, (1, REP))).astype(BF16)

    # rotate-half permutation: rot[d] = x[(d+HALF) % D] (sign lives in sin
    # tables).  As matmul lhsT: rotPm[k, d] = 1 iff k == (d+HALF) % D.
    rotPm = np.zeros((D, D), f32)
    rotPm[(np.arange(D) + HALF) % D, np.arange(D)] = 1.0
    rotPm = np.ascontiguousarray(rotPm).astype(BF16)

    qw = np.asarray(q_norm_w, f32).reshape(D)
    kw = np.asarray(k_norm_w, f32).reshape(D)
    rot_src = (np.arange(D) + HALF) % D                   # rotate-half source

    cosqT = np.asarray(cos_q, f32)[0, 0].T * SCALE * qw[:, None]
    sinqT = np.asarray(sin_q, f32)[0, 0].T.copy()
    sinqT[:HALF] = -sinqT[:HALF]
    sinqT = sinqT * SCALE * qw[rot_src][:, None]
    coskT = np.asarray(cos_k, f32)[0, 0].T * kw[:, None]
    sinkT = np.asarray(sin_k, f32)[0, 0].T.copy()
    sinkT[:HALF] = -sinkT[:HALF]
    sinkT = sinkT * kw[rot_src][:, None]

    Wq = np.asarray(Wq, f32)
    Wk = np.asarray(Wk, f32)
    Wv = np.asarray(Wv, f32)
    Wo = np.asarray(Wo, f32)
    kv = np.asarray(kv_cache, f32)

    in_maps = []
    for cidx in range(NCORES):
        hd = slice(cidx * REP * D, (cidx + 1) * REP * D)
        wq_c = Wq[hd].reshape(REP, D, HID)
        wq_c = wq_c - wq_c.mean(axis=1, keepdims=True)    # fold mean-subtract
        wq_c = wq_c.reshape(REP * D, HID)
        wk_c = Wk[cidx * D:(cidx + 1) * D]
        wk_c = wk_c - wk_c.mean(axis=0, keepdims=True)
        wv_c = Wv[cidx * D:(cidx + 1) * D]
        wkvT = np.concatenate([wk_c.T, wv_c.T], axis=1)   # [HID, 256]
        # packed [p, k, n]: wkvP[p, k, n] = wkvT[k*128+p, n]
        wkvP = np.ascontiguousarray(
            wkvT.reshape(KT, 128, 256).transpose(1, 0, 2)).astype(BF16)
        wqTc = np.ascontiguousarray(wq_c.T)               # [HID, 512]
        wqP = np.ascontiguousarray(
            wqTc.reshape(KT, 128, 512).transpose(1, 0, 2)).astype(BF16)
        # wo packed [p, h, e_chunk, 512]: woP[p,h,j,e'] = Wo.T[h*128+p, j*512+e']
        woTc = Wo[:, hd].T.reshape(REP, 128, HID // 512, 512)
        woP = np.ascontiguousarray(woTc.transpose(1, 0, 2, 3))
        ktold = np.ascontiguousarray(kv[0, cidx, T:, :].T)  # [D, SOLD]
        # vold packed [s_local, (tile d)]: voldP[p, n*128+d] = v[n*128+p, d]
        voldP = np.ascontiguousarray(
            kv[1, cidx, T:, :].reshape(SOLD // 128, 128, D)
            .transpose(1, 0, 2).reshape(128, SOLD))
        in_maps.append(dict(
            cTP=cTP,
            wkvP=wkvP,
            wqP=wqP.astype(BF16),
            xTp=xTp,
            woP=woP.astype(BF16),
            ktold=ktold.astype(BF16),
            voldP=voldP.astype(BF16),
            identb2=np.eye(128, dtype=f32).astype(BF16),
            rotP=rotPm,
            triP=triP,
            cosq=cosqT.astype(BF16), sinq=sinqT.astype(BF16),
            cosk=coskT.astype(BF16), sink=sinkT.astype(BF16),
        ))
    return in_maps


def kernel(**inputs) -> np.ndarray:
    global LAST_RESULTS
    if "nc" not in _PROGRAM_CACHE:
        _PROGRAM_CACHE["nc"] = _build_program()
    nc = _PROGRAM_CACHE["nc"]
    in_maps = _prepare_inputs(**inputs)
    trace = bool(int(os.environ.get("BASS_KERNEL_TRACE", "0")))
    res = run_bass_kernel_spmd(nc, in_maps, list(range(NCORES)), trace=trace)
    LAST_RESULTS = res
    y = np.zeros((L, HID), np.float64)
    for cidx in range(NCORES):
        y += res.results[cidx]["y"].astype(np.float64)
    return y.astype(np.float32).reshape(1, L, HID)
